# revision 54
# baseline (speedup 1.0000x reference)
"""Self-contained Trainium2 Bass kernel for the multi-head attention module.

Sharding: flat 8-way head tensor-parallelism. Core c owns heads {2c, 2c+1}
for both batches; after attention one 8-core AllToAll per head-pair index
reshards from head-space to sequence-space and each core runs the output
projection for its 512 token rows. Host concatenates the per-core row
chunks.

Layout: everything bf16 on the matmul paths (1 cyc/row on PE, half the
DMA + collective bytes). x is transposed on the host so the kernel DMAs
[D, T] tiles straight into SBUF: no PE transposes, no staging copies. V is
computed directly in [token, v] layout via xT-stationary matmuls. The
Activation engine runs only the softmax exps (it is the attention-phase
floor at ~1038ns per 256-key block vs the PE's 854ns); all PSUM->SBUF
copies live on DVE. Each attention unit is software-pipelined with scores
running two key-blocks ahead of the AV matmuls, and the projection /
output-pass matmuls are rationed into the ~184ns/block PE slack through a
filler queue so the Activation engine never starves. The output projection
is split into an even-heads pass (hidden behind late attention, after the
first AllToAll) and an odd-heads pass (the only work after the second
AllToAll).
"""

import sys

sys.path.insert(0, "/opt/trn_rl_repo")

from collections import deque

import ml_dtypes
import numpy as np

from concourse import bacc, bass_utils, mybir, tile

B, S, D, H, DK, DV, DO = 2, 2048, 1024, 16, 64, 64, 1024
T = B * S          # 4096 flattened tokens
NCORES = 8
HPC = H // NCORES  # 2 heads per core
ROWS = T // NCORES # 512 output rows per core
TCH = 512          # token chunk for projections / q chunks
F32 = mybir.dt.float32
F32R = mybir.dt.float32r
BF16 = mybir.dt.bfloat16
EXP = mybir.ActivationFunctionType.Exp

_cache = {}


def _build(collective=True):
    nc = bacc.Bacc("TRN2", target_bir_lowering=False, debug=False,
                   num_devices=NCORES if collective else 1)
    xt_d = nc.dram_tensor("xt", [D, T], BF16, kind="ExternalInput").ap()
    # host-pre-shuffled [128, 3*8*128]: wqkv[p, n*1024 + dc*128 + e] =
    # W_n[dc*128 + p, e] -- one fat contiguous DMA, no per-d-block strides
    wqkv_d = nc.dram_tensor("wqkv", [128, 3 * 8 * 128], BF16,
                            kind="ExternalInput").ap()
    wo_d = nc.dram_tensor("wo", [H * DV, DO], BF16, kind="ExternalInput").ap()
    out_d = nc.dram_tensor("out", [ROWS, DO], BF16, kind="ExternalOutput").ap()
    bnc_in0 = nc.dram_tensor("bnc_in0", [NCORES, 64, ROWS], BF16).ap()
    bnc_out0 = nc.dram_tensor("bnc_out0", [NCORES, 64, ROWS], BF16).ap()
    # the second head-pair's resharding is split into two row-half
    # collectives so the output projection can start on the first half
    # while the second is still in flight
    bnc_in1 = [nc.dram_tensor(f"bnc_in1_{r}", [NCORES, 64, ROWS // 2],
                              BF16).ap() for r in range(2)]
    bnc_out1 = [nc.dram_tensor(f"bnc_out1_{r}", [NCORES, 64, ROWS // 2],
                               BF16).ap() for r in range(2)]

    with tile.TileContext(nc) as tc:
        with (
            tc.tile_pool(name="sb", bufs=1) as sb,
            tc.tile_pool(name="ps", bufs=1, space="PSUM") as ps,
            nc.allow_low_precision(reason="bf16 compute is intentional"),
        ):
            # constants for the softmax-normalization broadcast matmul
            ones_f = sb.tile([128, 64], F32, tag="onesf", bufs=1)
            nc.vector.memset(ones_f[:], 1.0)
            ones_b = sb.tile([128, 64], F32R, tag="ones", bufs=1)
            nc.vector.tensor_copy(ones_b[:], ones_f[:])

            # HWDGE descriptor generation costs ~625ns per DMA instruction,
            # serialized, so inputs are fetched with as few fat strided DMAs
            # as possible. The first x chunk is interleaved with the weights
            # so phase 1 can start ~4us in.
            # qkv weights: one contiguous DMA for all three matrices
            # (host-pre-shuffled into d-block column layout)
            wqkv_sb = sb.tile([128, 3 * 8 * 128], BF16, tag="wqkv", bufs=1)
            _wn = {"q": 0, "k": 1, "v": 2}

            def w_slice(name, dc):
                c0 = _wn[name] * 1024 + dc * 128
                return wqkv_sb[:, c0:c0 + 128]

            # x^T: one [128, 8*512] tile per chunk (column blocks are the 8
            # d-blocks), loaded in two half DMAs (xc[p, dc*512+t] =
            # xt_d[dc*128+p, c0+t]).
            xTc = [sb.tile([128, 8 * TCH], BF16, tag="xTc", bufs=8,
                           name=f"xTc{tci}") for tci in range(8)]

            def load_x_chunk(tci, half, width=4):
                c0 = tci * TCH
                dc0 = half * width
                nc.sync.dma_start(
                    xTc[tci][:, dc0 * TCH:(dc0 + width) * TCH].rearrange(
                        "p (dc t) -> p dc t", t=TCH),
                    xt_d[dc0 * 128:(dc0 + width) * 128,
                         c0:c0 + TCH].rearrange(
                        "(dc p) t -> p dc t", p=128))

            # q weights first (phase 1 starts with the q projection), then
            # the first x chunk in quarters so its first d-blocks land
            # early, then the rest in halves
            nc.sync.dma_start(wqkv_sb[:, 0:1024], wqkv_d[:, 0:1024])
            for quarter in range(2):
                load_x_chunk(0, quarter, width=2)
            nc.sync.dma_start(wqkv_sb[:, 1024:3072], wqkv_d[:, 1024:3072])
            for quarter in range(2, 4):
                load_x_chunk(0, quarter, width=2)
            for tci in range(1, 8):
                load_x_chunk(tci, 0)
                load_x_chunk(tci, 1)

            def xT(dc, tci):
                return xTc[tci][:, dc * TCH:(dc + 1) * TCH]

            # wo pair tiles for the two projection passes: pass h reads heads
            # {4p+h, 4p+2+h} stacked on partitions, matching the oTf layout
            wo_p = {0: [], 1: []}
            for h in range(HPC):
                for p in range(4):
                    wt = sb.tile([128, DO], BF16, tag="wo", bufs=8,
                                 name=f"wo{h}_{p}")
                    for half, head in ((0, 4 * p + h), (1, 4 * p + 2 + h)):
                        nc.sync.dma_start(
                            wt[half * 64:half * 64 + 64, :],
                            wo_d[head * 64:head * 64 + 64, :])
                    wo_p[h].append(wt)

            # persistent activations
            qT = sb.tile([128, T], BF16, tag="qT", bufs=1)
            kT = sb.tile([128, T], BF16, tag="kT", bufs=1)
            # v in natural [token, v] layout: 32 t-blocks x (2 heads x
            # [64 v cols | ones]) -> AV stationary slices [128, 65]
            v_dual = sb.tile([128, 32 * 130], BF16, tag="vdual", bufs=1)
            ones_cols = v_dual[:].rearrange(
                "p (b h c) -> p b h c", h=2, c=65)[:, :, :, 64:65]
            nc.vector.memset(ones_cols, 1.0)

            last_obc = [None]

            # ---- filler queue: small PE thunks rationed into the
            # Act-bound slack of the attention inner loop ----
            fill_q = deque()  # entries: (cost_ns, label, thunk)

            def filler_slot(budget=200):
                spent = 0
                while fill_q and spent < budget:
                    cost, _, thunk = fill_q.popleft()
                    thunk()
                    spent += cost

            def flush_through(label):
                while any(e[1] == label for e in fill_q):
                    _, _, thunk = fill_q.popleft()
                    thunk()

            def flush_all():
                while fill_q:
                    fill_q.popleft()[2]()

            # ---- phase 1 parts: q/k (W stationary) and v (xT stationary)
            def qk_part_thunks(tci, name):
                holder = {}
                c0 = tci * TCH

                def mk(dc):
                    def t():
                        if dc == 0:
                            holder["pp"] = ps.tile(
                                [128, TCH], F32, tag="ps_a", bufs=2,
                                name=f"pp{tci}_{name}")
                        nc.tensor.matmul(
                            holder["pp"][:], w_slice(name, dc),
                            xT(dc, tci), start=(dc == 0), stop=(dc == 7))
                        if dc == 7:
                            dst = qT if name == "q" else kT
                            nc.vector.tensor_copy(dst[:, c0:c0 + TCH],
                                                  holder["pp"][:])
                    return t
                return [(213, f"{name}{tci}", mk(dc)) for dc in range(8)]

            def v_part_thunks(tci):
                holder = {}

                def mk(tb, dc):
                    def t():
                        if tb == 0 and dc == 0:
                            holder["pv"] = ps.tile(
                                [128, TCH], F32, tag="ps_a", bufs=2,
                                name=f"pv{tci}")
                        nc.tensor.matmul(
                            holder["pv"][:, tb * 128:(tb + 1) * 128],
                            xTc[tci][:, dc * TCH + tb * 128:
                                      dc * TCH + (tb + 1) * 128],
                            w_slice("v", dc),
                            start=(dc == 0), stop=(dc == 7))
                        if tb == 3 and dc == 7:
                            vd = v_dual[:, tci * 4 * 130:
                                        (tci + 1) * 4 * 130].rearrange(
                                "p (b h c) -> p b h c", h=2, c=65)[
                                :, :, :, 0:64]
                            nc.vector.tensor_copy(
                                vd, holder["pv"][:].rearrange(
                                    "p (b h c) -> p b h c", h=2, c=64))
                    return t
                return [(60, f"v{tci}", mk(tb, dc))
                        for tb in range(4) for dc in range(8)]

            def emit_proj(tci, which):
                # bulk emission (used for the pre-attention chunks); q
                # first to match the input DMA queue order
                if "q" in which:
                    for e in qk_part_thunks(tci, "q"):
                        e[2]()
                if "k" in which:
                    for e in qk_part_thunks(tci, "k"):
                        e[2]()
                if "v" in which:
                    for e in v_part_thunks(tci):
                        e[2]()

            # ---- attention unit (batch, head, q-chunk), software-pipelined:
            # scores+exp run two key-blocks ahead of the AV matmuls, filler
            # thunks absorb the ~184ns/block PE slack, and the
            # normalization tail is deferred into the next unit so the PE
            # never head-of-line blocks on the DVE reciprocal.
            pending_tail = [None]

            def emit_tail():
                if pending_tail[0] is None:
                    return
                b, h, qc, po, r65 = pending_tail[0]
                pending_tail[0] = None
                pbc = ps.tile([64, TCH], F32, tag="ps_s", bufs=2,
                              name=f"pbc{b}_{h}_{qc}")
                nc.tensor.matmul(pbc[:], ones_b[64:65, :],
                                 r65[64:65, :], start=True, stop=True)
                bc_sb = sb.tile([64, TCH], F32R, tag="bcsb", bufs=2,
                                name=f"bcsb{b}_{h}_{qc}")
                nc.vector.tensor_copy(bc_sb[:], pbc[:])
                obc = sb.tile([64, TCH], BF16, tag="obc", bufs=3,
                              name=f"obc{b}_{h}_{qc}")
                nc.vector.tensor_mul(obc[:], po[0:64, :], bc_sb[:])
                shard = b * (S // TCH) + qc
                if h == 0:
                    nc.sync.dma_start(bnc_in0[shard, :, :], obc[:])
                else:
                    for r in range(2):
                        nc.sync.dma_start(
                            bnc_in1[r][shard, :, :],
                            obc[:, r * (TCH // 2):(r + 1) * (TCH // 2)])
                last_obc[0] = obc

            # The 16 attention units run as one globally software-pipelined
            # stream: the scores+exp block always runs exactly two
            # key-blocks ahead of the AV matmuls (which matches the 2-deep
            # ps_s rotation: a new pscr's slot belongs to the block whose
            # ex the PE just consumed), so the PE never waits on an exp --
            # not even across unit boundaries.
            sched = ([(0, h, qc) for h in range(HPC)
                      for qc in range(S // TCH)]
                     + [(1, 0, qc) for qc in range(S // TCH)]
                     + [(1, 1, qc) for qc in range(S // TCH)])
            ex_store = {}
            pre_s_hook = {}

            def emit_s(u, i):
                if (u, i) in pre_s_hook:
                    pre_s_hook.pop((u, i))()
                b, h, qc = sched[u]
                qoff = b * S + qc * TCH
                pscr = ps.tile([128, 2 * TCH], F32, tag="ps_s", bufs=2,
                               name=f"pscr{u}_{i}")
                for j in range(2):
                    koff = b * S + (2 * i + j) * 128
                    nc.tensor.matmul(
                        pscr[:, j * TCH:(j + 1) * TCH],
                        kT[h * 64:(h + 1) * 64, koff:koff + 128],
                        qT[h * 64:(h + 1) * 64, qoff:qoff + TCH],
                        start=True, stop=True)
                ex = sb.tile([128, 2 * TCH], BF16, tag="ex", bufs=4,
                             name=f"ex{u}_{i}")
                nc.scalar.activation(ex[:], pscr[:], EXP, scale=0.125)
                ex_store[(u, i)] = ex

            at_unit_start = {}

            def run_units():
                emit_s(0, 0)
                emit_s(0, 1)
                for u in range(len(sched)):
                    b, h, qc = sched[u]
                    po = ps.tile([65, TCH], F32, tag="ps_o", bufs=2,
                                 name=f"po{u}")
                    for i in range(8):
                        ex = ex_store.pop((u, i))
                        for j in range(2):
                            kb = 2 * i + j
                            blk = b * 16 + kb
                            nc.tensor.matmul(
                                po[:],
                                v_dual[:, blk * 130 + h * 65:
                                       blk * 130 + h * 65 + 65],
                                ex[:, j * TCH:(j + 1) * TCH],
                                start=(kb == 0), stop=(kb == S // 128 - 1))
                        if i + 2 < 8:
                            emit_s(u, i + 2)
                        elif u + 1 < len(sched):
                            emit_s(u + 1, i - 6)
                        if i == 0:
                            # deferred normalization of the previous unit:
                            # by now that pscr slot's exp is done, so the
                            # ps_s rotation slot for pbc is free
                            emit_tail()
                            if u in at_unit_start:
                                at_unit_start.pop(u)()
                        filler_slot()
                    # reciprocal right at unit end so the deferred pbc
                    # matmul never waits on the DVE
                    r65 = sb.tile([65, TCH], F32R, tag="r", bufs=2,
                                  name=f"r{u}")
                    nc.vector.reciprocal(r65[64:65, :], po[64:65, :])
                    pending_tail[0] = (b, h, qc, po, r65)

            def emit_a2a0():
                if collective:
                    nc.gpsimd.collective_compute(
                        "AllToAll", mybir.AluOpType.bypass,
                        replica_groups=[list(range(NCORES))],
                        ins=[bnc_in0[:]], outs=[bnc_out0[:]])
                else:
                    nc.sync.dma_start(bnc_out0[:], bnc_in0[:])

            def emit_a2a1(r):
                if collective:
                    nc.gpsimd.collective_compute(
                        "AllToAll", mybir.AluOpType.bypass,
                        replica_groups=[list(range(NCORES))],
                        ins=[bnc_in1[r][:]], outs=[bnc_out1[r][:]])
                else:
                    nc.sync.dma_start(bnc_out1[r][:], bnc_in1[r][:])

            # ---- output projection pass h: heads {4p+h, 4p+2+h};
            # oTf[h][j*64+r, p*512+t] = bnc_out[h][2p+j, r, t]
            oTf = {}

            def emit_oTf0():
                t = sb.tile([128, 4 * ROWS], BF16, tag="oTf", bufs=2,
                            name="oTf0")
                for j in range(2):
                    nc.sync.dma_start(
                        t[64 * j:64 * j + 64, :].rearrange(
                            "r (p tt) -> r p tt", tt=ROWS),
                        bnc_out0[:].rearrange(
                            "(p j) r tt -> j r p tt", j=2)[j])
                oTf[0] = t

            def emit_oTf1(r):
                if 1 not in oTf:
                    oTf[1] = sb.tile([128, 4 * ROWS], BF16, tag="oTf",
                                     bufs=2, name="oTf1")
                t = oTf[1]
                hw = ROWS // 2
                for j in range(2):
                    nc.sync.dma_start(
                        t[64 * j:64 * j + 64, :].rearrange(
                            "rr (p tt) -> rr p tt", tt=ROWS)[
                            :, :, r * hw:(r + 1) * hw],
                        bnc_out1[r][:].rearrange(
                            "(p j) rr tt -> j rr p tt", j=2)[j])

            o0sb = [sb.tile([128, 512], BF16, tag="o0sb", bufs=8,
                            name=f"o0sb{i}") for i in range(8)]

            def pass0_thunks(ci):
                sbi, doc = divmod(ci, 2)
                holder = {}

                def mk(p):
                    def t():
                        if p == 0:
                            holder["pout"] = ps.tile(
                                [128, 512], F32, tag="ps_a", bufs=2,
                                name=f"p0_{ci}")
                        nc.tensor.matmul(
                            holder["pout"][:],
                            oTf[0][:, p * ROWS + sbi * 128:
                                   p * ROWS + (sbi + 1) * 128],
                            wo_p[0][p][:, doc * 512:(doc + 1) * 512],
                            start=(p == 0), stop=(p == 3))
                        if p == 3:
                            nc.vector.tensor_copy(o0sb[ci][:],
                                                  holder["pout"][:])
                    return t
                return [(213, f"p0_{ci}", mk(p)) for p in range(4)]

            def emit_pass1_half(r):
                for sbi in (2 * r, 2 * r + 1):
                    outt = sb.tile([128, DO], BF16, tag="osb", bufs=2,
                                   name=f"outt{sbi}")
                    for doc in range(2):
                        pout = ps.tile([128, 512], F32, tag="ps_a", bufs=2,
                                       name=f"p1_{sbi}_{doc}")
                        for p in range(4):
                            nc.tensor.matmul(
                                pout[:],
                                oTf[1][:, p * ROWS + sbi * 128:
                                       p * ROWS + (sbi + 1) * 128],
                                wo_p[1][p][:, doc * 512:(doc + 1) * 512],
                                start=(p == 0), stop=(p == 3))
                        nc.vector.tensor_add(
                            outt[:, doc * 512:(doc + 1) * 512], pout[:],
                            o0sb[sbi * 2 + doc][:])
                        # ship each half as soon as its add lands so only
                        # the last 128KB DMA is exposed at the end
                        nc.sync.dma_start(
                            out_d[sbi * 128:(sbi + 1) * 128,
                                  doc * 512:(doc + 1) * 512],
                            outt[:, doc * 512:(doc + 1) * 512])

            # ---- schedule ----
            # Warm the PE clock while the first DMAs land (f32r with a
            # 64-wide moving AP runs at 4 cyc/row: ~394ns per warm matmul).
            for wi in range(12):
                wps = ps.tile([64, 64], F32, tag="ps_s", bufs=2,
                              name=f"swarm{wi}")
                nc.tensor.matmul(wps[:], ones_b[0:1, :], ones_b[0:1, :],
                                 start=True, stop=True)
            # chunks 0-1 in bulk, then attention starts (score block i of a
            # batch-0 unit only needs kT chunk i//2); chunks 2-7 and the
            # deferred q parts ration through the filler queue, with
            # just-in-time flush hooks at the score blocks that need them.
            for tci in range(2):
                emit_proj(tci, "qkv")
            for tci in range(2, 8):
                fill_q.extend(qk_part_thunks(tci, "k"))
                fill_q.extend(v_part_thunks(tci))
                fill_q.extend(qk_part_thunks(tci, "q"))

            # units 0 and 8 meet their later key chunks mid-stream (score
            # block i needs kT chunk i//2 resp. 4+i//2, AV block i needs
            # the matching v blocks); other units need their own q chunk
            # before their first scores
            def _fl(label):
                return lambda: flush_through(label)

            pre_s_hook[(0, 4)] = _fl("v2")
            pre_s_hook[(0, 6)] = _fl("v3")
            pre_s_hook[(2, 0)] = _fl("q2")
            pre_s_hook[(3, 0)] = _fl("q3")
            for qc in range(S // TCH):
                pre_s_hook[(8 + qc, 0)] = _fl(f"q{4 + qc}")

            def h0_done():
                # unit 11's obc has just been flushed by emit_tail above
                emit_a2a0()
                emit_oTf0()
            at_unit_start[12] = h0_done

            run_units()
            emit_tail()  # flush (1,1,3)'s obc before the h1 collectives
            flush_all()
            emit_a2a1(0)
            emit_a2a1(1)
            # the whole even-heads projection pass runs inside the h1
            # collective window (it only needs oTf0/wo): the first h1 half
            # collective hides behind it, the second behind pass1-half0;
            # a few warms keep the PE clock pegged through the residue
            for ci in range(8):
                for e in pass0_thunks(ci):
                    e[2]()
            for wi in range(8):
                wps = ps.tile([64, TCH], F32, tag="ps_s", bufs=2,
                              name=f"warm{wi}")
                nc.tensor.matmul(
                    wps[:], o0sb[0][:, 0:64], o0sb[0][:],
                    start=True, stop=True)
            emit_oTf1(0)
            emit_oTf1(1)
            emit_pass1_half(0)
            emit_pass1_half(1)

    nc.compile()
    return nc


def _get_nc():
    if "nc" not in _cache:
        _cache["nc"] = _build()
    return _cache["nc"]


def _dshuffle(w):
    # [D, 128] -> [128, 8*128] with out[p, dc*128+e] = w[dc*128+p, e]
    return w.reshape(8, 128, 128).transpose(1, 0, 2).reshape(128, 1024)


def _in_maps(x, Wq, Wk, Wv, Wo):
    bf16 = ml_dtypes.bfloat16
    xt = np.ascontiguousarray(
        x.reshape(T, D).T.astype(bf16))
    wo = np.ascontiguousarray(Wo.astype(bf16))
    maps = []
    for c in range(NCORES):
        h0, h1 = HPC * c, HPC * c + 1
        wqkv = np.concatenate(
            [_dshuffle(np.concatenate([W[h0], W[h1]], axis=1))
             for W in (Wq, Wk, Wv)], axis=1)
        maps.append({
            "xt": xt,
            "wqkv": np.ascontiguousarray(wqkv.astype(bf16)),
            "wo": wo,
        })
    return maps


def kernel(x, Wq, Wk, Wv, Wo, **_):
    nc = _get_nc()
    res = bass_utils.run_bass_kernel_spmd(
        nc, _in_maps(x, Wq, Wk, Wv, Wo), core_ids=list(range(NCORES)))
    out = np.concatenate(
        [res.results[c]["out"].astype(np.float32) for c in range(NCORES)],
        axis=0)
    return out.reshape(B, S, DO)


# revision 56
# speedup vs baseline: 1.0114x; 1.0114x over previous
"""Self-contained Trainium2 Bass kernel for the multi-head attention module.

Sharding: flat 8-way head tensor-parallelism. Core c owns heads {2c, 2c+1}
for both batches; after attention one 8-core AllToAll per head-pair index
reshards from head-space to sequence-space and each core runs the output
projection for its 512 token rows. Host concatenates the per-core row
chunks.

Layout: everything bf16 on the matmul paths (1 cyc/row on PE, half the
DMA + collective bytes). x is transposed on the host so the kernel DMAs
[D, T] tiles straight into SBUF: no PE transposes, no staging copies. V is
computed directly in [token, v] layout via xT-stationary matmuls. The
Activation engine runs only the softmax exps (it is the attention-phase
floor at ~1038ns per 256-key block vs the PE's 854ns); all PSUM->SBUF
copies live on DVE. Each attention unit is software-pipelined with scores
running two key-blocks ahead of the AV matmuls, and the projection /
output-pass matmuls are rationed into the ~184ns/block PE slack through a
filler queue so the Activation engine never starves. The output projection
is split into an even-heads pass (hidden behind late attention, after the
first AllToAll) and an odd-heads pass (the only work after the second
AllToAll).
"""

import sys

sys.path.insert(0, "/opt/trn_rl_repo")

from collections import deque

import ml_dtypes
import numpy as np

from concourse import bacc, bass_utils, mybir, tile

B, S, D, H, DK, DV, DO = 2, 2048, 1024, 16, 64, 64, 1024
T = B * S          # 4096 flattened tokens
NCORES = 8
HPC = H // NCORES  # 2 heads per core
ROWS = T // NCORES # 512 output rows per core
TCH = 512          # token chunk for projections / q chunks
F32 = mybir.dt.float32
F32R = mybir.dt.float32r
BF16 = mybir.dt.bfloat16
EXP = mybir.ActivationFunctionType.Exp

_cache = {}


def _build(collective=True):
    nc = bacc.Bacc("TRN2", target_bir_lowering=False, debug=False,
                   num_devices=NCORES if collective else 1)
    xt_d = nc.dram_tensor("xt", [D, T], BF16, kind="ExternalInput").ap()
    # host-pre-shuffled [128, 3*8*128]: wqkv[p, n*1024 + dc*128 + e] =
    # W_n[dc*128 + p, e] -- one fat contiguous DMA, no per-d-block strides
    wqkv_d = nc.dram_tensor("wqkv", [128, 3 * 8 * 128], BF16,
                            kind="ExternalInput").ap()
    wo_d = nc.dram_tensor("wo", [H * DV, DO], BF16, kind="ExternalInput").ap()
    out_d = nc.dram_tensor("out", [ROWS, DO], BF16, kind="ExternalOutput").ap()
    bnc_in0 = nc.dram_tensor("bnc_in0", [NCORES, 64, ROWS], BF16).ap()
    bnc_out0 = nc.dram_tensor("bnc_out0", [NCORES, 64, ROWS], BF16).ap()
    # the second head-pair's resharding is split into two row-half
    # collectives so the output projection can start on the first half
    # while the second is still in flight
    bnc_in1 = [nc.dram_tensor(f"bnc_in1_{r}", [NCORES, 64, ROWS // 2],
                              BF16).ap() for r in range(2)]
    bnc_out1 = [nc.dram_tensor(f"bnc_out1_{r}", [NCORES, 64, ROWS // 2],
                               BF16).ap() for r in range(2)]

    with tile.TileContext(nc) as tc:
        with (
            tc.tile_pool(name="sb", bufs=1) as sb,
            tc.tile_pool(name="ps", bufs=1, space="PSUM") as ps,
            nc.allow_low_precision(reason="bf16 compute is intentional"),
        ):
            # constants for the softmax-normalization broadcast matmul
            ones_f = sb.tile([128, 64], F32, tag="onesf", bufs=1)
            nc.vector.memset(ones_f[:], 1.0)
            ones_b = sb.tile([128, 64], F32R, tag="ones", bufs=1)
            nc.vector.tensor_copy(ones_b[:], ones_f[:])

            # HWDGE descriptor generation costs ~625ns per DMA instruction,
            # serialized, so inputs are fetched with as few fat strided DMAs
            # as possible. The first x chunk is interleaved with the weights
            # so phase 1 can start ~4us in.
            # qkv weights: one contiguous DMA for all three matrices
            # (host-pre-shuffled into d-block column layout)
            wqkv_sb = sb.tile([128, 3 * 8 * 128], BF16, tag="wqkv", bufs=1)
            _wn = {"q": 0, "k": 1, "v": 2}

            def w_slice(name, dc):
                c0 = _wn[name] * 1024 + dc * 128
                return wqkv_sb[:, c0:c0 + 128]

            # x^T: one [128, 8*512] tile per chunk (column blocks are the 8
            # d-blocks), loaded in two half DMAs (xc[p, dc*512+t] =
            # xt_d[dc*128+p, c0+t]).
            xTc = [sb.tile([128, 8 * TCH], BF16, tag="xTc", bufs=8,
                           name=f"xTc{tci}") for tci in range(8)]

            def load_x_chunk(tci, half, width=4):
                c0 = tci * TCH
                dc0 = half * width
                nc.sync.dma_start(
                    xTc[tci][:, dc0 * TCH:(dc0 + width) * TCH].rearrange(
                        "p (dc t) -> p dc t", t=TCH),
                    xt_d[dc0 * 128:(dc0 + width) * 128,
                         c0:c0 + TCH].rearrange(
                        "(dc p) t -> p dc t", p=128))

            # q weights first (phase 1 starts with the q projection), then
            # the first x chunk in quarters so its first d-blocks land
            # early, then the rest in halves
            nc.sync.dma_start(wqkv_sb[:, 0:1024], wqkv_d[:, 0:1024])
            for quarter in range(2):
                load_x_chunk(0, quarter, width=2)
            nc.sync.dma_start(wqkv_sb[:, 1024:3072], wqkv_d[:, 1024:3072])
            for quarter in range(2, 4):
                load_x_chunk(0, quarter, width=2)
            for tci in range(1, 8):
                load_x_chunk(tci, 0)
                load_x_chunk(tci, 1)

            def xT(dc, tci):
                return xTc[tci][:, dc * TCH:(dc + 1) * TCH]

            # wo pair tiles for the two projection passes: pass h reads heads
            # {4p+h, 4p+2+h} stacked on partitions, matching the oTf layout
            wo_p = {0: [], 1: []}
            for h in range(HPC):
                for p in range(4):
                    wt = sb.tile([128, DO], BF16, tag="wo", bufs=8,
                                 name=f"wo{h}_{p}")
                    for half, head in ((0, 4 * p + h), (1, 4 * p + 2 + h)):
                        nc.sync.dma_start(
                            wt[half * 64:half * 64 + 64, :],
                            wo_d[head * 64:head * 64 + 64, :])
                    wo_p[h].append(wt)

            # persistent activations
            qT = sb.tile([128, T], BF16, tag="qT", bufs=1)
            kT = sb.tile([128, T], BF16, tag="kT", bufs=1)
            # v in natural [token, v] layout: 32 t-blocks x (2 heads x
            # [64 v cols | ones]) -> AV stationary slices [128, 65]
            v_dual = sb.tile([128, 32 * 130], BF16, tag="vdual", bufs=1)
            ones_cols = v_dual[:].rearrange(
                "p (b h c) -> p b h c", h=2, c=65)[:, :, :, 64:65]
            nc.vector.memset(ones_cols, 1.0)

            last_obc = [None]

            # ---- filler queue: small PE thunks rationed into the
            # Act-bound slack of the attention inner loop ----
            fill_q = deque()  # entries: (cost_ns, label, thunk)

            def filler_slot(budget=200):
                spent = 0
                while fill_q and spent < budget:
                    cost, _, thunk = fill_q.popleft()
                    thunk()
                    spent += cost

            def flush_through(label):
                while any(e[1] == label for e in fill_q):
                    _, _, thunk = fill_q.popleft()
                    thunk()

            def flush_all():
                while fill_q:
                    fill_q.popleft()[2]()

            # ---- phase 1 parts: q/k (W stationary) and v (xT stationary)
            def qk_part_thunks(tci, name):
                holder = {}
                c0 = tci * TCH

                def mk(dc):
                    def t():
                        if dc == 0:
                            holder["pp"] = ps.tile(
                                [128, TCH], F32, tag="ps_a", bufs=2,
                                name=f"pp{tci}_{name}")
                        nc.tensor.matmul(
                            holder["pp"][:], w_slice(name, dc),
                            xT(dc, tci), start=(dc == 0), stop=(dc == 7))
                        if dc == 7:
                            dst = qT if name == "q" else kT
                            nc.vector.tensor_copy(dst[:, c0:c0 + TCH],
                                                  holder["pp"][:])
                    return t
                return [(213, f"{name}{tci}", mk(dc)) for dc in range(8)]

            def v_part_thunks(tci):
                holder = {}

                def mk(tb, dc):
                    def t():
                        if tb == 0 and dc == 0:
                            holder["pv"] = ps.tile(
                                [128, TCH], F32, tag="ps_a", bufs=2,
                                name=f"pv{tci}")
                        nc.tensor.matmul(
                            holder["pv"][:, tb * 128:(tb + 1) * 128],
                            xTc[tci][:, dc * TCH + tb * 128:
                                      dc * TCH + (tb + 1) * 128],
                            w_slice("v", dc),
                            start=(dc == 0), stop=(dc == 7))
                        if tb == 3 and dc == 7:
                            vd = v_dual[:, tci * 4 * 130:
                                        (tci + 1) * 4 * 130].rearrange(
                                "p (b h c) -> p b h c", h=2, c=65)[
                                :, :, :, 0:64]
                            nc.vector.tensor_copy(
                                vd, holder["pv"][:].rearrange(
                                    "p (b h c) -> p b h c", h=2, c=64))
                    return t
                return [(60, f"v{tci}", mk(tb, dc))
                        for tb in range(4) for dc in range(8)]

            def emit_proj(tci, which):
                # bulk emission (used for the pre-attention chunks); q
                # first to match the input DMA queue order
                if "q" in which:
                    for e in qk_part_thunks(tci, "q"):
                        e[2]()
                if "k" in which:
                    for e in qk_part_thunks(tci, "k"):
                        e[2]()
                if "v" in which:
                    for e in v_part_thunks(tci):
                        e[2]()

            # ---- attention unit (batch, head, q-chunk), software-pipelined:
            # scores+exp run two key-blocks ahead of the AV matmuls, filler
            # thunks absorb the ~184ns/block PE slack, and the
            # normalization tail is deferred into the next unit so the PE
            # never head-of-line blocks on the DVE reciprocal.
            pending_tail = [None]

            def emit_tail():
                if pending_tail[0] is None:
                    return
                b, h, qc, po, r65 = pending_tail[0]
                pending_tail[0] = None
                pbc = ps.tile([64, TCH], F32, tag="ps_s", bufs=2,
                              name=f"pbc{b}_{h}_{qc}")
                nc.tensor.matmul(pbc[:], ones_b[64:65, :],
                                 r65[64:65, :], start=True, stop=True)
                bc_sb = sb.tile([64, TCH], F32R, tag="bcsb", bufs=2,
                                name=f"bcsb{b}_{h}_{qc}")
                nc.vector.tensor_copy(bc_sb[:], pbc[:])
                obc = sb.tile([64, TCH], BF16, tag="obc", bufs=3,
                              name=f"obc{b}_{h}_{qc}")
                nc.vector.tensor_mul(obc[:], po[0:64, :], bc_sb[:])
                shard = b * (S // TCH) + qc
                if h == 0:
                    nc.sync.dma_start(bnc_in0[shard, :, :], obc[:])
                else:
                    for r in range(2):
                        nc.sync.dma_start(
                            bnc_in1[r][shard, :, :],
                            obc[:, r * (TCH // 2):(r + 1) * (TCH // 2)])
                last_obc[0] = obc

            # The 16 attention units run as one globally software-pipelined
            # stream: the scores+exp block always runs exactly two
            # key-blocks ahead of the AV matmuls (which matches the 2-deep
            # ps_s rotation: a new pscr's slot belongs to the block whose
            # ex the PE just consumed), so the PE never waits on an exp --
            # not even across unit boundaries.
            sched = ([(0, h, qc) for h in range(HPC)
                      for qc in range(S // TCH)]
                     + [(1, 0, qc) for qc in range(S // TCH)]
                     + [(1, 1, qc) for qc in range(S // TCH)])
            ex_store = {}
            pre_s_hook = {}

            def emit_s(u, i):
                if (u, i) in pre_s_hook:
                    pre_s_hook.pop((u, i))()
                b, h, qc = sched[u]
                qoff = b * S + qc * TCH
                pscr = ps.tile([128, 2 * TCH], F32, tag="ps_s", bufs=2,
                               name=f"pscr{u}_{i}")
                for j in range(2):
                    koff = b * S + (2 * i + j) * 128
                    nc.tensor.matmul(
                        pscr[:, j * TCH:(j + 1) * TCH],
                        kT[h * 64:(h + 1) * 64, koff:koff + 128],
                        qT[h * 64:(h + 1) * 64, qoff:qoff + TCH],
                        start=True, stop=True)
                ex = sb.tile([128, 2 * TCH], BF16, tag="ex", bufs=4,
                             name=f"ex{u}_{i}")
                nc.scalar.activation(ex[:], pscr[:], EXP, scale=0.125)
                ex_store[(u, i)] = ex

            at_unit_start = {}

            def run_units():
                emit_s(0, 0)
                emit_s(0, 1)
                for u in range(len(sched)):
                    b, h, qc = sched[u]
                    po = ps.tile([65, TCH], F32, tag="ps_o", bufs=2,
                                 name=f"po{u}")
                    for i in range(8):
                        ex = ex_store.pop((u, i))
                        for j in range(2):
                            kb = 2 * i + j
                            blk = b * 16 + kb
                            nc.tensor.matmul(
                                po[:],
                                v_dual[:, blk * 130 + h * 65:
                                       blk * 130 + h * 65 + 65],
                                ex[:, j * TCH:(j + 1) * TCH],
                                start=(kb == 0), stop=(kb == S // 128 - 1))
                        if i + 2 < 8:
                            emit_s(u, i + 2)
                        elif u + 1 < len(sched):
                            emit_s(u + 1, i - 6)
                        if i == 0:
                            # deferred normalization of the previous unit:
                            # by now that pscr slot's exp is done, so the
                            # ps_s rotation slot for pbc is free
                            emit_tail()
                            if u in at_unit_start:
                                at_unit_start.pop(u)()
                        filler_slot()
                    # reciprocal right at unit end so the deferred pbc
                    # matmul never waits on the DVE
                    r65 = sb.tile([65, TCH], F32R, tag="r", bufs=2,
                                  name=f"r{u}")
                    nc.vector.reciprocal(r65[64:65, :], po[64:65, :])
                    pending_tail[0] = (b, h, qc, po, r65)

            def emit_a2a0():
                if collective:
                    nc.gpsimd.collective_compute(
                        "AllToAll", mybir.AluOpType.bypass,
                        replica_groups=[list(range(NCORES))],
                        ins=[bnc_in0[:]], outs=[bnc_out0[:]])
                else:
                    nc.sync.dma_start(bnc_out0[:], bnc_in0[:])

            def emit_a2a1(r):
                if collective:
                    nc.gpsimd.collective_compute(
                        "AllToAll", mybir.AluOpType.bypass,
                        replica_groups=[list(range(NCORES))],
                        ins=[bnc_in1[r][:]], outs=[bnc_out1[r][:]])
                else:
                    nc.sync.dma_start(bnc_out1[r][:], bnc_in1[r][:])

            # ---- output projection pass h: heads {4p+h, 4p+2+h};
            # oTf[h][j*64+r, p*512+t] = bnc_out[h][2p+j, r, t]
            oTf = {}

            def emit_oTf0():
                t = sb.tile([128, 4 * ROWS], BF16, tag="oTf", bufs=2,
                            name="oTf0")
                for j in range(2):
                    nc.sync.dma_start(
                        t[64 * j:64 * j + 64, :].rearrange(
                            "r (p tt) -> r p tt", tt=ROWS),
                        bnc_out0[:].rearrange(
                            "(p j) r tt -> j r p tt", j=2)[j])
                oTf[0] = t

            def emit_oTf1(r):
                if 1 not in oTf:
                    oTf[1] = sb.tile([128, 4 * ROWS], BF16, tag="oTf",
                                     bufs=2, name="oTf1")
                t = oTf[1]
                hw = ROWS // 2
                for j in range(2):
                    nc.sync.dma_start(
                        t[64 * j:64 * j + 64, :].rearrange(
                            "rr (p tt) -> rr p tt", tt=ROWS)[
                            :, :, r * hw:(r + 1) * hw],
                        bnc_out1[r][:].rearrange(
                            "(p j) rr tt -> j rr p tt", j=2)[j])

            o0sb = [sb.tile([128, 512], BF16, tag="o0sb", bufs=8,
                            name=f"o0sb{i}") for i in range(8)]

            def pass0_thunks(ci):
                sbi, doc = divmod(ci, 2)
                holder = {}

                def mk(p):
                    def t():
                        if p == 0:
                            holder["pout"] = ps.tile(
                                [128, 512], F32, tag="ps_a", bufs=2,
                                name=f"p0_{ci}")
                        nc.tensor.matmul(
                            holder["pout"][:],
                            oTf[0][:, p * ROWS + sbi * 128:
                                   p * ROWS + (sbi + 1) * 128],
                            wo_p[0][p][:, doc * 512:(doc + 1) * 512],
                            start=(p == 0), stop=(p == 3))
                        if p == 3:
                            nc.vector.tensor_copy(o0sb[ci][:],
                                                  holder["pout"][:])
                    return t
                return [(213, f"p0_{ci}", mk(p)) for p in range(4)]

            def emit_pass1_half(r):
                for sbi in (2 * r, 2 * r + 1):
                    outt = sb.tile([128, DO], BF16, tag="osb", bufs=2,
                                   name=f"outt{sbi}")
                    for doc in range(2):
                        pout = ps.tile([128, 512], F32, tag="ps_a", bufs=2,
                                       name=f"p1_{sbi}_{doc}")
                        for p in range(4):
                            nc.tensor.matmul(
                                pout[:],
                                oTf[1][:, p * ROWS + sbi * 128:
                                       p * ROWS + (sbi + 1) * 128],
                                wo_p[1][p][:, doc * 512:(doc + 1) * 512],
                                start=(p == 0), stop=(p == 3))
                        nc.vector.tensor_add(
                            outt[:, doc * 512:(doc + 1) * 512], pout[:],
                            o0sb[sbi * 2 + doc][:])
                        # ship each half as soon as its add lands so only
                        # the last 128KB DMA is exposed at the end
                        nc.sync.dma_start(
                            out_d[sbi * 128:(sbi + 1) * 128,
                                  doc * 512:(doc + 1) * 512],
                            outt[:, doc * 512:(doc + 1) * 512])

            # ---- schedule ----
            # Warm the PE clock while the first DMAs land (f32r with a
            # 64-wide moving AP runs at 4 cyc/row: ~394ns per warm matmul).
            for wi in range(12):
                wps = ps.tile([64, 64], F32, tag="ps_s", bufs=2,
                              name=f"swarm{wi}")
                nc.tensor.matmul(wps[:], ones_b[0:1, :], ones_b[0:1, :],
                                 start=True, stop=True)
            # chunks 0-1 in bulk, then attention starts (score block i of a
            # batch-0 unit only needs kT chunk i//2); chunks 2-7 and the
            # deferred q parts ration through the filler queue, with
            # just-in-time flush hooks at the score blocks that need them.
            for tci in range(2):
                emit_proj(tci, "qkv")
            for tci in range(2, 8):
                fill_q.extend(qk_part_thunks(tci, "k"))
                fill_q.extend(v_part_thunks(tci))
                fill_q.extend(qk_part_thunks(tci, "q"))

            # units 0 and 8 meet their later key chunks mid-stream (score
            # block i needs kT chunk i//2 resp. 4+i//2, AV block i needs
            # the matching v blocks); other units need their own q chunk
            # before their first scores
            def _fl(label):
                return lambda: flush_through(label)

            pre_s_hook[(0, 4)] = _fl("v2")
            pre_s_hook[(0, 6)] = _fl("v3")
            pre_s_hook[(2, 0)] = _fl("q2")
            pre_s_hook[(3, 0)] = _fl("q3")
            for qc in range(S // TCH):
                pre_s_hook[(8 + qc, 0)] = _fl(f"q{4 + qc}")

            def h0_done():
                # unit 11's obc has just been flushed by emit_tail above
                emit_a2a0()
                emit_oTf0()
                for ci in range(8):
                    fill_q.extend(pass0_thunks(ci))
            at_unit_start[12] = h0_done

            run_units()
            emit_tail()  # flush (1,1,3)'s obc before the h1 collectives
            flush_all()
            emit_a2a1(0)
            emit_a2a1(1)
            # warms (on a long-ready operand) bridge the PE clock across
            # the h1 collective window so the odd-heads projection pass
            # starts at 2.4 GHz
            for wi in range(32):
                wps = ps.tile([64, TCH], F32, tag="ps_s", bufs=2,
                              name=f"warm{wi}")
                nc.tensor.matmul(
                    wps[:], o0sb[0][:, 0:64], o0sb[0][:],
                    start=True, stop=True)
            emit_oTf1(0)
            emit_oTf1(1)
            emit_pass1_half(0)
            emit_pass1_half(1)

    nc.compile()
    return nc


def _get_nc():
    if "nc" not in _cache:
        _cache["nc"] = _build()
    return _cache["nc"]


def _dshuffle(w):
    # [D, 128] -> [128, 8*128] with out[p, dc*128+e] = w[dc*128+p, e]
    return w.reshape(8, 128, 128).transpose(1, 0, 2).reshape(128, 1024)


def _in_maps(x, Wq, Wk, Wv, Wo):
    bf16 = ml_dtypes.bfloat16
    xt = np.ascontiguousarray(
        x.reshape(T, D).T.astype(bf16))
    wo = np.ascontiguousarray(Wo.astype(bf16))
    maps = []
    for c in range(NCORES):
        h0, h1 = HPC * c, HPC * c + 1
        wqkv = np.concatenate(
            [_dshuffle(np.concatenate([W[h0], W[h1]], axis=1))
             for W in (Wq, Wk, Wv)], axis=1)
        maps.append({
            "xt": xt,
            "wqkv": np.ascontiguousarray(wqkv.astype(bf16)),
            "wo": wo,
        })
    return maps


def kernel(x, Wq, Wk, Wv, Wo, **_):
    nc = _get_nc()
    res = bass_utils.run_bass_kernel_spmd(
        nc, _in_maps(x, Wq, Wk, Wv, Wo), core_ids=list(range(NCORES)))
    out = np.concatenate(
        [res.results[c]["out"].astype(np.float32) for c in range(NCORES)],
        axis=0)
    return out.reshape(B, S, DO)


# revision 57
# speedup vs baseline: 1.0155x; 1.0040x over previous
"""Self-contained Trainium2 Bass kernel for the multi-head attention module.

Sharding: flat 8-way head tensor-parallelism. Core c owns heads {2c, 2c+1}
for both batches; after attention one 8-core AllToAll per head-pair index
reshards from head-space to sequence-space and each core runs the output
projection for its 512 token rows. Host concatenates the per-core row
chunks.

Layout: everything bf16 on the matmul paths (1 cyc/row on PE, half the
DMA + collective bytes). x is transposed on the host so the kernel DMAs
[D, T] tiles straight into SBUF: no PE transposes, no staging copies. V is
computed directly in [token, v] layout via xT-stationary matmuls. The
Activation engine runs only the softmax exps (it is the attention-phase
floor at ~1038ns per 256-key block vs the PE's 854ns); all PSUM->SBUF
copies live on DVE. Each attention unit is software-pipelined with scores
running two key-blocks ahead of the AV matmuls, and the projection /
output-pass matmuls are rationed into the ~184ns/block PE slack through a
filler queue so the Activation engine never starves. The output projection
is split into an even-heads pass (hidden behind late attention, after the
first AllToAll) and an odd-heads pass (the only work after the second
AllToAll).
"""

import sys

sys.path.insert(0, "/opt/trn_rl_repo")

from collections import deque

import ml_dtypes
import numpy as np

from concourse import bacc, bass_utils, mybir, tile

B, S, D, H, DK, DV, DO = 2, 2048, 1024, 16, 64, 64, 1024
T = B * S          # 4096 flattened tokens
NCORES = 8
HPC = H // NCORES  # 2 heads per core
ROWS = T // NCORES # 512 output rows per core
TCH = 512          # token chunk for projections / q chunks
F32 = mybir.dt.float32
F32R = mybir.dt.float32r
BF16 = mybir.dt.bfloat16
EXP = mybir.ActivationFunctionType.Exp

_cache = {}


def _build(collective=True):
    nc = bacc.Bacc("TRN2", target_bir_lowering=False, debug=False,
                   num_devices=NCORES if collective else 1)
    xt_d = nc.dram_tensor("xt", [D, T], BF16, kind="ExternalInput").ap()
    # host-pre-shuffled [128, 3*8*128]: wqkv[p, n*1024 + dc*128 + e] =
    # W_n[dc*128 + p, e] -- one fat contiguous DMA, no per-d-block strides
    wqkv_d = nc.dram_tensor("wqkv", [128, 3 * 8 * 128], BF16,
                            kind="ExternalInput").ap()
    wo_d = nc.dram_tensor("wo", [H * DV, DO], BF16, kind="ExternalInput").ap()
    out_d = nc.dram_tensor("out", [ROWS, DO], BF16, kind="ExternalOutput").ap()
    bnc_in0 = nc.dram_tensor("bnc_in0", [NCORES, 64, ROWS], BF16).ap()
    bnc_out0 = nc.dram_tensor("bnc_out0", [NCORES, 64, ROWS], BF16).ap()
    # the second head-pair's resharding is split into two row-half
    # collectives so the output projection can start on the first half
    # while the second is still in flight
    bnc_in1 = [nc.dram_tensor(f"bnc_in1_{r}", [NCORES, 64, ROWS // 2],
                              BF16).ap() for r in range(2)]
    bnc_out1 = [nc.dram_tensor(f"bnc_out1_{r}", [NCORES, 64, ROWS // 2],
                               BF16).ap() for r in range(2)]

    with tile.TileContext(nc) as tc:
        with (
            tc.tile_pool(name="sb", bufs=1) as sb,
            tc.tile_pool(name="ps", bufs=1, space="PSUM") as ps,
            nc.allow_low_precision(reason="bf16 compute is intentional"),
        ):
            # constants for the softmax-normalization broadcast matmul
            ones_f = sb.tile([128, 64], F32, tag="onesf", bufs=1)
            nc.vector.memset(ones_f[:], 1.0)
            ones_b = sb.tile([128, 64], F32R, tag="ones", bufs=1)
            nc.vector.tensor_copy(ones_b[:], ones_f[:])

            # HWDGE descriptor generation costs ~625ns per DMA instruction,
            # serialized, so inputs are fetched with as few fat strided DMAs
            # as possible. The first x chunk is interleaved with the weights
            # so phase 1 can start ~4us in.
            # qkv weights: one contiguous DMA for all three matrices
            # (host-pre-shuffled into d-block column layout)
            wqkv_sb = sb.tile([128, 3 * 8 * 128], BF16, tag="wqkv", bufs=1)
            _wn = {"q": 0, "k": 1, "v": 2}

            def w_slice(name, dc):
                c0 = _wn[name] * 1024 + dc * 128
                return wqkv_sb[:, c0:c0 + 128]

            # x^T: one [128, 8*512] tile per chunk (column blocks are the 8
            # d-blocks), loaded in two half DMAs (xc[p, dc*512+t] =
            # xt_d[dc*128+p, c0+t]).
            xTc = [sb.tile([128, 8 * TCH], BF16, tag="xTc", bufs=8,
                           name=f"xTc{tci}") for tci in range(8)]

            def load_x_chunk(tci, half, width=4):
                c0 = tci * TCH
                dc0 = half * width
                nc.sync.dma_start(
                    xTc[tci][:, dc0 * TCH:(dc0 + width) * TCH].rearrange(
                        "p (dc t) -> p dc t", t=TCH),
                    xt_d[dc0 * 128:(dc0 + width) * 128,
                         c0:c0 + TCH].rearrange(
                        "(dc p) t -> p dc t", p=128))

            # q weights first (phase 1 starts with the q projection), then
            # the first x chunk in quarters so its first d-blocks land
            # early, then the rest in halves
            nc.sync.dma_start(wqkv_sb[:, 0:1024], wqkv_d[:, 0:1024])
            for quarter in range(2):
                load_x_chunk(0, quarter, width=2)
            nc.sync.dma_start(wqkv_sb[:, 1024:3072], wqkv_d[:, 1024:3072])
            for quarter in range(2, 4):
                load_x_chunk(0, quarter, width=2)
            for tci in range(1, 8):
                load_x_chunk(tci, 0)
                load_x_chunk(tci, 1)

            def xT(dc, tci):
                return xTc[tci][:, dc * TCH:(dc + 1) * TCH]

            # wo pair tiles for the two projection passes: pass h reads heads
            # {4p+h, 4p+2+h} stacked on partitions, matching the oTf layout
            wo_p = {0: [], 1: []}
            for h in range(HPC):
                for p in range(4):
                    wt = sb.tile([128, DO], BF16, tag="wo", bufs=8,
                                 name=f"wo{h}_{p}")
                    for half, head in ((0, 4 * p + h), (1, 4 * p + 2 + h)):
                        nc.sync.dma_start(
                            wt[half * 64:half * 64 + 64, :],
                            wo_d[head * 64:head * 64 + 64, :])
                    wo_p[h].append(wt)

            # persistent activations
            qT = sb.tile([128, T], BF16, tag="qT", bufs=1)
            kT = sb.tile([128, T], BF16, tag="kT", bufs=1)
            # v in natural [token, v] layout: 32 t-blocks x (2 heads x
            # [64 v cols | ones]) -> AV stationary slices [128, 65]
            v_dual = sb.tile([128, 32 * 130], BF16, tag="vdual", bufs=1)
            ones_cols = v_dual[:].rearrange(
                "p (b h c) -> p b h c", h=2, c=65)[:, :, :, 64:65]
            nc.vector.memset(ones_cols, 1.0)

            last_obc = [None]

            # ---- filler queue: small PE thunks rationed into the
            # Act-bound slack of the attention inner loop ----
            fill_q = deque()  # entries: (cost_ns, label, thunk)

            def filler_slot(budget=250):
                spent = 0
                while fill_q and spent < budget:
                    cost, _, thunk = fill_q.popleft()
                    thunk()
                    spent += cost

            def flush_through(label):
                while any(e[1] == label for e in fill_q):
                    _, _, thunk = fill_q.popleft()
                    thunk()

            def flush_all():
                while fill_q:
                    fill_q.popleft()[2]()

            # ---- phase 1 parts: q/k (W stationary) and v (xT stationary)
            def qk_part_thunks(tci, name):
                holder = {}
                c0 = tci * TCH

                def mk(dc):
                    def t():
                        if dc == 0:
                            holder["pp"] = ps.tile(
                                [128, TCH], F32, tag="ps_a", bufs=2,
                                name=f"pp{tci}_{name}")
                        nc.tensor.matmul(
                            holder["pp"][:], w_slice(name, dc),
                            xT(dc, tci), start=(dc == 0), stop=(dc == 7))
                        if dc == 7:
                            dst = qT if name == "q" else kT
                            nc.vector.tensor_copy(dst[:, c0:c0 + TCH],
                                                  holder["pp"][:])
                    return t
                return [(213, f"{name}{tci}", mk(dc)) for dc in range(8)]

            def v_part_thunks(tci):
                holder = {}

                def mk(tb, dc):
                    def t():
                        if tb == 0 and dc == 0:
                            holder["pv"] = ps.tile(
                                [128, TCH], F32, tag="ps_a", bufs=2,
                                name=f"pv{tci}")
                        nc.tensor.matmul(
                            holder["pv"][:, tb * 128:(tb + 1) * 128],
                            xTc[tci][:, dc * TCH + tb * 128:
                                      dc * TCH + (tb + 1) * 128],
                            w_slice("v", dc),
                            start=(dc == 0), stop=(dc == 7))
                        if tb == 3 and dc == 7:
                            vd = v_dual[:, tci * 4 * 130:
                                        (tci + 1) * 4 * 130].rearrange(
                                "p (b h c) -> p b h c", h=2, c=65)[
                                :, :, :, 0:64]
                            nc.vector.tensor_copy(
                                vd, holder["pv"][:].rearrange(
                                    "p (b h c) -> p b h c", h=2, c=64))
                    return t
                return [(60, f"v{tci}", mk(tb, dc))
                        for tb in range(4) for dc in range(8)]

            def emit_proj(tci, which):
                # bulk emission (used for the pre-attention chunks); q
                # first to match the input DMA queue order
                if "q" in which:
                    for e in qk_part_thunks(tci, "q"):
                        e[2]()
                if "k" in which:
                    for e in qk_part_thunks(tci, "k"):
                        e[2]()
                if "v" in which:
                    for e in v_part_thunks(tci):
                        e[2]()

            # ---- attention unit (batch, head, q-chunk), software-pipelined:
            # scores+exp run two key-blocks ahead of the AV matmuls, filler
            # thunks absorb the ~184ns/block PE slack, and the
            # normalization tail is deferred into the next unit so the PE
            # never head-of-line blocks on the DVE reciprocal.
            pending_tail = [None]

            def emit_tail():
                if pending_tail[0] is None:
                    return
                b, h, qc, po, r65 = pending_tail[0]
                pending_tail[0] = None
                pbc = ps.tile([64, TCH], F32, tag="ps_s", bufs=2,
                              name=f"pbc{b}_{h}_{qc}")
                nc.tensor.matmul(pbc[:], ones_b[64:65, :],
                                 r65[64:65, :], start=True, stop=True)
                bc_sb = sb.tile([64, TCH], F32R, tag="bcsb", bufs=2,
                                name=f"bcsb{b}_{h}_{qc}")
                nc.vector.tensor_copy(bc_sb[:], pbc[:])
                obc = sb.tile([64, TCH], BF16, tag="obc", bufs=3,
                              name=f"obc{b}_{h}_{qc}")
                nc.vector.tensor_mul(obc[:], po[0:64, :], bc_sb[:])
                shard = b * (S // TCH) + qc
                if h == 0:
                    nc.sync.dma_start(bnc_in0[shard, :, :], obc[:])
                else:
                    for r in range(2):
                        nc.sync.dma_start(
                            bnc_in1[r][shard, :, :],
                            obc[:, r * (TCH // 2):(r + 1) * (TCH // 2)])
                last_obc[0] = obc

            # The 16 attention units run as one globally software-pipelined
            # stream: the scores+exp block always runs exactly two
            # key-blocks ahead of the AV matmuls (which matches the 2-deep
            # ps_s rotation: a new pscr's slot belongs to the block whose
            # ex the PE just consumed), so the PE never waits on an exp --
            # not even across unit boundaries.
            sched = ([(0, h, qc) for h in range(HPC)
                      for qc in range(S // TCH)]
                     + [(1, 0, qc) for qc in range(S // TCH)]
                     + [(1, 1, qc) for qc in range(S // TCH)])
            ex_store = {}
            pre_s_hook = {}

            def emit_s(u, i):
                if (u, i) in pre_s_hook:
                    pre_s_hook.pop((u, i))()
                b, h, qc = sched[u]
                qoff = b * S + qc * TCH
                pscr = ps.tile([128, 2 * TCH], F32, tag="ps_s", bufs=2,
                               name=f"pscr{u}_{i}")
                for j in range(2):
                    koff = b * S + (2 * i + j) * 128
                    nc.tensor.matmul(
                        pscr[:, j * TCH:(j + 1) * TCH],
                        kT[h * 64:(h + 1) * 64, koff:koff + 128],
                        qT[h * 64:(h + 1) * 64, qoff:qoff + TCH],
                        start=True, stop=True)
                ex = sb.tile([128, 2 * TCH], BF16, tag="ex", bufs=4,
                             name=f"ex{u}_{i}")
                nc.scalar.activation(ex[:], pscr[:], EXP, scale=0.125)
                ex_store[(u, i)] = ex

            at_unit_start = {}

            def run_units():
                emit_s(0, 0)
                emit_s(0, 1)
                for u in range(len(sched)):
                    b, h, qc = sched[u]
                    po = ps.tile([65, TCH], F32, tag="ps_o", bufs=2,
                                 name=f"po{u}")
                    for i in range(8):
                        ex = ex_store.pop((u, i))
                        for j in range(2):
                            kb = 2 * i + j
                            blk = b * 16 + kb
                            nc.tensor.matmul(
                                po[:],
                                v_dual[:, blk * 130 + h * 65:
                                       blk * 130 + h * 65 + 65],
                                ex[:, j * TCH:(j + 1) * TCH],
                                start=(kb == 0), stop=(kb == S // 128 - 1))
                        if i + 2 < 8:
                            emit_s(u, i + 2)
                        elif u + 1 < len(sched):
                            emit_s(u + 1, i - 6)
                        if i == 0:
                            # deferred normalization of the previous unit:
                            # by now that pscr slot's exp is done, so the
                            # ps_s rotation slot for pbc is free
                            emit_tail()
                            if u in at_unit_start:
                                at_unit_start.pop(u)()
                        filler_slot()
                    # reciprocal right at unit end so the deferred pbc
                    # matmul never waits on the DVE
                    r65 = sb.tile([65, TCH], F32R, tag="r", bufs=2,
                                  name=f"r{u}")
                    nc.vector.reciprocal(r65[64:65, :], po[64:65, :])
                    pending_tail[0] = (b, h, qc, po, r65)

            def emit_a2a0():
                if collective:
                    nc.gpsimd.collective_compute(
                        "AllToAll", mybir.AluOpType.bypass,
                        replica_groups=[list(range(NCORES))],
                        ins=[bnc_in0[:]], outs=[bnc_out0[:]])
                else:
                    nc.sync.dma_start(bnc_out0[:], bnc_in0[:])

            def emit_a2a1(r):
                if collective:
                    nc.gpsimd.collective_compute(
                        "AllToAll", mybir.AluOpType.bypass,
                        replica_groups=[list(range(NCORES))],
                        ins=[bnc_in1[r][:]], outs=[bnc_out1[r][:]])
                else:
                    nc.sync.dma_start(bnc_out1[r][:], bnc_in1[r][:])

            # ---- output projection pass h: heads {4p+h, 4p+2+h};
            # oTf[h][j*64+r, p*512+t] = bnc_out[h][2p+j, r, t]
            oTf = {}

            def emit_oTf0():
                t = sb.tile([128, 4 * ROWS], BF16, tag="oTf", bufs=2,
                            name="oTf0")
                for j in range(2):
                    nc.sync.dma_start(
                        t[64 * j:64 * j + 64, :].rearrange(
                            "r (p tt) -> r p tt", tt=ROWS),
                        bnc_out0[:].rearrange(
                            "(p j) r tt -> j r p tt", j=2)[j])
                oTf[0] = t

            def emit_oTf1(r):
                if 1 not in oTf:
                    oTf[1] = sb.tile([128, 4 * ROWS], BF16, tag="oTf",
                                     bufs=2, name="oTf1")
                t = oTf[1]
                hw = ROWS // 2
                for j in range(2):
                    nc.sync.dma_start(
                        t[64 * j:64 * j + 64, :].rearrange(
                            "rr (p tt) -> rr p tt", tt=ROWS)[
                            :, :, r * hw:(r + 1) * hw],
                        bnc_out1[r][:].rearrange(
                            "(p j) rr tt -> j rr p tt", j=2)[j])

            o0sb = [sb.tile([128, 512], BF16, tag="o0sb", bufs=8,
                            name=f"o0sb{i}") for i in range(8)]

            def pass0_thunks(ci):
                sbi, doc = divmod(ci, 2)
                holder = {}

                def mk(p):
                    def t():
                        if p == 0:
                            holder["pout"] = ps.tile(
                                [128, 512], F32, tag="ps_a", bufs=2,
                                name=f"p0_{ci}")
                        nc.tensor.matmul(
                            holder["pout"][:],
                            oTf[0][:, p * ROWS + sbi * 128:
                                   p * ROWS + (sbi + 1) * 128],
                            wo_p[0][p][:, doc * 512:(doc + 1) * 512],
                            start=(p == 0), stop=(p == 3))
                        if p == 3:
                            nc.vector.tensor_copy(o0sb[ci][:],
                                                  holder["pout"][:])
                    return t
                return [(213, f"p0_{ci}", mk(p)) for p in range(4)]

            def emit_pass1_half(r):
                for sbi in (2 * r, 2 * r + 1):
                    outt = sb.tile([128, DO], BF16, tag="osb", bufs=2,
                                   name=f"outt{sbi}")
                    for doc in range(2):
                        pout = ps.tile([128, 512], F32, tag="ps_a", bufs=2,
                                       name=f"p1_{sbi}_{doc}")
                        for p in range(4):
                            nc.tensor.matmul(
                                pout[:],
                                oTf[1][:, p * ROWS + sbi * 128:
                                       p * ROWS + (sbi + 1) * 128],
                                wo_p[1][p][:, doc * 512:(doc + 1) * 512],
                                start=(p == 0), stop=(p == 3))
                        nc.vector.tensor_add(
                            outt[:, doc * 512:(doc + 1) * 512], pout[:],
                            o0sb[sbi * 2 + doc][:])
                        # ship each half as soon as its add lands so only
                        # the last 128KB DMA is exposed at the end
                        nc.sync.dma_start(
                            out_d[sbi * 128:(sbi + 1) * 128,
                                  doc * 512:(doc + 1) * 512],
                            outt[:, doc * 512:(doc + 1) * 512])

            # ---- schedule ----
            # Warm the PE clock while the first DMAs land (f32r with a
            # 64-wide moving AP runs at 4 cyc/row: ~394ns per warm matmul).
            for wi in range(12):
                wps = ps.tile([64, 64], F32, tag="ps_s", bufs=2,
                              name=f"swarm{wi}")
                nc.tensor.matmul(wps[:], ones_b[0:1, :], ones_b[0:1, :],
                                 start=True, stop=True)
            # chunks 0-1 in bulk, then attention starts (score block i of a
            # batch-0 unit only needs kT chunk i//2); chunks 2-7 and the
            # deferred q parts ration through the filler queue, with
            # just-in-time flush hooks at the score blocks that need them.
            for tci in range(2):
                emit_proj(tci, "qkv")
            for tci in range(2, 8):
                fill_q.extend(qk_part_thunks(tci, "k"))
                fill_q.extend(v_part_thunks(tci))
                fill_q.extend(qk_part_thunks(tci, "q"))

            # units 0 and 8 meet their later key chunks mid-stream (score
            # block i needs kT chunk i//2 resp. 4+i//2, AV block i needs
            # the matching v blocks); other units need their own q chunk
            # before their first scores
            def _fl(label):
                return lambda: flush_through(label)

            pre_s_hook[(0, 4)] = _fl("v2")
            pre_s_hook[(0, 6)] = _fl("v3")
            pre_s_hook[(2, 0)] = _fl("q2")
            pre_s_hook[(3, 0)] = _fl("q3")
            for qc in range(S // TCH):
                pre_s_hook[(8 + qc, 0)] = _fl(f"q{4 + qc}")

            def h0_done():
                # unit 11's obc has just been flushed by emit_tail above
                emit_a2a0()
                emit_oTf0()
                for ci in range(8):
                    fill_q.extend(pass0_thunks(ci))
            at_unit_start[12] = h0_done

            run_units()
            emit_tail()  # flush (1,1,3)'s obc before the h1 collectives
            flush_all()
            emit_a2a1(0)
            emit_a2a1(1)
            # warms (on a long-ready operand) bridge the PE clock across
            # the h1 collective window so the odd-heads projection pass
            # starts at 2.4 GHz
            for wi in range(32):
                wps = ps.tile([64, TCH], F32, tag="ps_s", bufs=2,
                              name=f"warm{wi}")
                nc.tensor.matmul(
                    wps[:], o0sb[0][:, 0:64], o0sb[0][:],
                    start=True, stop=True)
            emit_oTf1(0)
            emit_oTf1(1)
            emit_pass1_half(0)
            emit_pass1_half(1)

    nc.compile()
    return nc


def _get_nc():
    if "nc" not in _cache:
        _cache["nc"] = _build()
    return _cache["nc"]


def _dshuffle(w):
    # [D, 128] -> [128, 8*128] with out[p, dc*128+e] = w[dc*128+p, e]
    return w.reshape(8, 128, 128).transpose(1, 0, 2).reshape(128, 1024)


def _in_maps(x, Wq, Wk, Wv, Wo):
    bf16 = ml_dtypes.bfloat16
    xt = np.ascontiguousarray(
        x.reshape(T, D).T.astype(bf16))
    wo = np.ascontiguousarray(Wo.astype(bf16))
    maps = []
    for c in range(NCORES):
        h0, h1 = HPC * c, HPC * c + 1
        wqkv = np.concatenate(
            [_dshuffle(np.concatenate([W[h0], W[h1]], axis=1))
             for W in (Wq, Wk, Wv)], axis=1)
        maps.append({
            "xt": xt,
            "wqkv": np.ascontiguousarray(wqkv.astype(bf16)),
            "wo": wo,
        })
    return maps


def kernel(x, Wq, Wk, Wv, Wo, **_):
    nc = _get_nc()
    res = bass_utils.run_bass_kernel_spmd(
        nc, _in_maps(x, Wq, Wk, Wv, Wo), core_ids=list(range(NCORES)))
    out = np.concatenate(
        [res.results[c]["out"].astype(np.float32) for c in range(NCORES)],
        axis=0)
    return out.reshape(B, S, DO)


# revision 58
# speedup vs baseline: 1.0247x; 1.0091x over previous
"""Self-contained Trainium2 Bass kernel for the multi-head attention module.

Sharding: flat 8-way head tensor-parallelism. Core c owns heads {2c, 2c+1}
for both batches; after attention one 8-core AllToAll per head-pair index
reshards from head-space to sequence-space and each core runs the output
projection for its 512 token rows. Host concatenates the per-core row
chunks.

Layout: everything bf16 on the matmul paths (1 cyc/row on PE, half the
DMA + collective bytes). x is transposed on the host so the kernel DMAs
[D, T] tiles straight into SBUF: no PE transposes, no staging copies. V is
computed directly in [token, v] layout via xT-stationary matmuls. The
Activation engine runs only the softmax exps (it is the attention-phase
floor at ~1038ns per 256-key block vs the PE's 854ns); all PSUM->SBUF
copies live on DVE. Each attention unit is software-pipelined with scores
running two key-blocks ahead of the AV matmuls, and the projection /
output-pass matmuls are rationed into the ~184ns/block PE slack through a
filler queue so the Activation engine never starves. The output projection
is split into an even-heads pass (hidden behind late attention, after the
first AllToAll) and an odd-heads pass (the only work after the second
AllToAll).
"""

import sys

sys.path.insert(0, "/opt/trn_rl_repo")

from collections import deque

import ml_dtypes
import numpy as np

from concourse import bacc, bass_utils, mybir, tile

B, S, D, H, DK, DV, DO = 2, 2048, 1024, 16, 64, 64, 1024
T = B * S          # 4096 flattened tokens
NCORES = 8
HPC = H // NCORES  # 2 heads per core
ROWS = T // NCORES # 512 output rows per core
TCH = 512          # token chunk for projections / q chunks
F32 = mybir.dt.float32
F32R = mybir.dt.float32r
BF16 = mybir.dt.bfloat16
EXP = mybir.ActivationFunctionType.Exp

_cache = {}


def _build(collective=True):
    nc = bacc.Bacc("TRN2", target_bir_lowering=False, debug=False,
                   num_devices=NCORES if collective else 1)
    xt_d = nc.dram_tensor("xt", [D, T], BF16, kind="ExternalInput").ap()
    # host-pre-shuffled [128, 3*8*128]: wqkv[p, n*1024 + dc*128 + e] =
    # W_n[dc*128 + p, e] -- one fat contiguous DMA, no per-d-block strides
    wqkv_d = nc.dram_tensor("wqkv", [128, 3 * 8 * 128], BF16,
                            kind="ExternalInput").ap()
    wo_d = nc.dram_tensor("wo", [H * DV, DO], BF16, kind="ExternalInput").ap()
    out_d = nc.dram_tensor("out", [ROWS, DO], BF16, kind="ExternalOutput").ap()
    bnc_in0 = nc.dram_tensor("bnc_in0", [NCORES, 64, ROWS], BF16).ap()
    bnc_out0 = nc.dram_tensor("bnc_out0", [NCORES, 64, ROWS], BF16).ap()
    # the second head-pair's resharding is split into two row-half
    # collectives so the output projection can start on the first half
    # while the second is still in flight
    bnc_in1 = [nc.dram_tensor(f"bnc_in1_{r}", [NCORES, 64, ROWS // 2],
                              BF16).ap() for r in range(2)]
    bnc_out1 = [nc.dram_tensor(f"bnc_out1_{r}", [NCORES, 64, ROWS // 2],
                               BF16).ap() for r in range(2)]

    with tile.TileContext(nc) as tc:
        with (
            tc.tile_pool(name="sb", bufs=1) as sb,
            tc.tile_pool(name="ps", bufs=1, space="PSUM") as ps,
            nc.allow_low_precision(reason="bf16 compute is intentional"),
        ):
            # constants for the softmax-normalization broadcast matmul
            ones_f = sb.tile([128, 64], F32, tag="onesf", bufs=1)
            nc.vector.memset(ones_f[:], 1.0)
            ones_b = sb.tile([128, 64], F32R, tag="ones", bufs=1)
            nc.vector.tensor_copy(ones_b[:], ones_f[:])

            # HWDGE descriptor generation costs ~625ns per DMA instruction,
            # serialized, so inputs are fetched with as few fat strided DMAs
            # as possible. The first x chunk is interleaved with the weights
            # so phase 1 can start ~4us in.
            # qkv weights: one contiguous DMA for all three matrices
            # (host-pre-shuffled into d-block column layout)
            wqkv_sb = sb.tile([128, 3 * 8 * 128], BF16, tag="wqkv", bufs=1)
            _wn = {"q": 0, "k": 1, "v": 2}

            def w_slice(name, dc):
                c0 = _wn[name] * 1024 + dc * 128
                return wqkv_sb[:, c0:c0 + 128]

            # x^T: one [128, 8*512] tile per chunk (column blocks are the 8
            # d-blocks), loaded in two half DMAs (xc[p, dc*512+t] =
            # xt_d[dc*128+p, c0+t]).
            xTc = [sb.tile([128, 8 * TCH], BF16, tag="xTc", bufs=8,
                           name=f"xTc{tci}") for tci in range(8)]

            def load_x_chunk(tci, half, width=4):
                c0 = tci * TCH
                dc0 = half * width
                nc.sync.dma_start(
                    xTc[tci][:, dc0 * TCH:(dc0 + width) * TCH].rearrange(
                        "p (dc t) -> p dc t", t=TCH),
                    xt_d[dc0 * 128:(dc0 + width) * 128,
                         c0:c0 + TCH].rearrange(
                        "(dc p) t -> p dc t", p=128))

            # q weights first (phase 1 starts with the q projection), then
            # the first x chunk in quarters so its first d-blocks land
            # early, then the rest in halves
            nc.sync.dma_start(wqkv_sb[:, 0:1024], wqkv_d[:, 0:1024])
            for quarter in range(2):
                load_x_chunk(0, quarter, width=2)
            nc.sync.dma_start(wqkv_sb[:, 1024:3072], wqkv_d[:, 1024:3072])
            for quarter in range(2, 4):
                load_x_chunk(0, quarter, width=2)
            for tci in range(1, 8):
                load_x_chunk(tci, 0)
                load_x_chunk(tci, 1)

            def xT(dc, tci):
                return xTc[tci][:, dc * TCH:(dc + 1) * TCH]

            # wo pair tiles for the two projection passes: pass h reads heads
            # {4p+h, 4p+2+h} stacked on partitions, matching the oTf layout
            wo_p = {0: [], 1: []}
            for h in range(HPC):
                for p in range(4):
                    wt = sb.tile([128, DO], BF16, tag="wo", bufs=8,
                                 name=f"wo{h}_{p}")
                    for half, head in ((0, 4 * p + h), (1, 4 * p + 2 + h)):
                        nc.sync.dma_start(
                            wt[half * 64:half * 64 + 64, :],
                            wo_d[head * 64:head * 64 + 64, :])
                    wo_p[h].append(wt)

            # persistent activations
            qT = sb.tile([128, T], BF16, tag="qT", bufs=1)
            kT = sb.tile([128, T], BF16, tag="kT", bufs=1)
            # v in natural [token, v] layout: 32 t-blocks x (2 heads x
            # [64 v cols | ones]) -> AV stationary slices [128, 65]
            v_dual = sb.tile([128, 32 * 130], BF16, tag="vdual", bufs=1)
            ones_cols = v_dual[:].rearrange(
                "p (b h c) -> p b h c", h=2, c=65)[:, :, :, 64:65]
            nc.vector.memset(ones_cols, 1.0)

            last_obc = [None]

            # ---- filler queue: small PE thunks rationed into the
            # Act-bound slack of the attention inner loop ----
            fill_q = deque()  # entries: (cost_ns, label, thunk)

            def filler_slot(budget=170):
                spent = 0
                while fill_q and spent < budget:
                    cost, _, thunk = fill_q.popleft()
                    thunk()
                    spent += cost

            def flush_through(label):
                while any(e[1] == label for e in fill_q):
                    _, _, thunk = fill_q.popleft()
                    thunk()

            def flush_all():
                while fill_q:
                    fill_q.popleft()[2]()

            # ---- phase 1 parts: q/k (W stationary) and v (xT stationary)
            def qk_part_thunks(tci, name):
                holder = {}
                c0 = tci * TCH

                def mk(dc):
                    def t():
                        if dc == 0:
                            holder["pp"] = ps.tile(
                                [128, TCH], F32, tag="ps_a", bufs=2,
                                name=f"pp{tci}_{name}")
                        nc.tensor.matmul(
                            holder["pp"][:], w_slice(name, dc),
                            xT(dc, tci), start=(dc == 0), stop=(dc == 7))
                        if dc == 7:
                            dst = qT if name == "q" else kT
                            nc.vector.tensor_copy(dst[:, c0:c0 + TCH],
                                                  holder["pp"][:])
                    return t
                return [(213, f"{name}{tci}", mk(dc)) for dc in range(8)]

            def v_part_thunks(tci):
                holder = {}

                def mk(tb, dc):
                    def t():
                        if tb == 0 and dc == 0:
                            holder["pv"] = ps.tile(
                                [128, TCH], F32, tag="ps_a", bufs=2,
                                name=f"pv{tci}")
                        nc.tensor.matmul(
                            holder["pv"][:, tb * 128:(tb + 1) * 128],
                            xTc[tci][:, dc * TCH + tb * 128:
                                      dc * TCH + (tb + 1) * 128],
                            w_slice("v", dc),
                            start=(dc == 0), stop=(dc == 7))
                        if tb == 3 and dc == 7:
                            vd = v_dual[:, tci * 4 * 130:
                                        (tci + 1) * 4 * 130].rearrange(
                                "p (b h c) -> p b h c", h=2, c=65)[
                                :, :, :, 0:64]
                            nc.vector.tensor_copy(
                                vd, holder["pv"][:].rearrange(
                                    "p (b h c) -> p b h c", h=2, c=64))
                    return t
                return [(60, f"v{tci}", mk(tb, dc))
                        for tb in range(4) for dc in range(8)]

            def emit_proj(tci, which):
                # bulk emission (used for the pre-attention chunks); q
                # first to match the input DMA queue order
                if "q" in which:
                    for e in qk_part_thunks(tci, "q"):
                        e[2]()
                if "k" in which:
                    for e in qk_part_thunks(tci, "k"):
                        e[2]()
                if "v" in which:
                    for e in v_part_thunks(tci):
                        e[2]()

            # ---- attention unit (batch, head, q-chunk), software-pipelined:
            # scores+exp run two key-blocks ahead of the AV matmuls, filler
            # thunks absorb the ~184ns/block PE slack, and the
            # normalization tail is deferred into the next unit so the PE
            # never head-of-line blocks on the DVE reciprocal.
            pending_tail = [None]

            def emit_tail():
                if pending_tail[0] is None:
                    return
                b, h, qc, po, r65 = pending_tail[0]
                pending_tail[0] = None
                pbc = ps.tile([64, TCH], F32, tag="ps_s", bufs=2,
                              name=f"pbc{b}_{h}_{qc}")
                nc.tensor.matmul(pbc[:], ones_b[64:65, :],
                                 r65[64:65, :], start=True, stop=True)
                bc_sb = sb.tile([64, TCH], F32R, tag="bcsb", bufs=2,
                                name=f"bcsb{b}_{h}_{qc}")
                nc.vector.tensor_copy(bc_sb[:], pbc[:])
                obc = sb.tile([64, TCH], BF16, tag="obc", bufs=3,
                              name=f"obc{b}_{h}_{qc}")
                nc.vector.tensor_mul(obc[:], po[0:64, :], bc_sb[:])
                shard = b * (S // TCH) + qc
                if h == 0:
                    nc.sync.dma_start(bnc_in0[shard, :, :], obc[:])
                else:
                    for r in range(2):
                        nc.sync.dma_start(
                            bnc_in1[r][shard, :, :],
                            obc[:, r * (TCH // 2):(r + 1) * (TCH // 2)])
                last_obc[0] = obc

            # The 16 attention units run as one globally software-pipelined
            # stream: the scores+exp block always runs exactly two
            # key-blocks ahead of the AV matmuls (which matches the 2-deep
            # ps_s rotation: a new pscr's slot belongs to the block whose
            # ex the PE just consumed), so the PE never waits on an exp --
            # not even across unit boundaries.
            sched = ([(0, h, qc) for h in range(HPC)
                      for qc in range(S // TCH)]
                     + [(1, 0, qc) for qc in range(S // TCH)]
                     + [(1, 1, qc) for qc in range(S // TCH)])
            ex_store = {}
            pre_s_hook = {}

            def emit_s(u, i):
                if (u, i) in pre_s_hook:
                    pre_s_hook.pop((u, i))()
                b, h, qc = sched[u]
                qoff = b * S + qc * TCH
                pscr = ps.tile([128, 2 * TCH], F32, tag="ps_s", bufs=2,
                               name=f"pscr{u}_{i}")
                for j in range(2):
                    koff = b * S + (2 * i + j) * 128
                    nc.tensor.matmul(
                        pscr[:, j * TCH:(j + 1) * TCH],
                        kT[h * 64:(h + 1) * 64, koff:koff + 128],
                        qT[h * 64:(h + 1) * 64, qoff:qoff + TCH],
                        start=True, stop=True)
                ex = sb.tile([128, 2 * TCH], BF16, tag="ex", bufs=4,
                             name=f"ex{u}_{i}")
                nc.scalar.activation(ex[:], pscr[:], EXP, scale=0.125)
                ex_store[(u, i)] = ex

            at_unit_start = {}

            def run_units():
                emit_s(0, 0)
                emit_s(0, 1)
                for u in range(len(sched)):
                    b, h, qc = sched[u]
                    po = ps.tile([65, TCH], F32, tag="ps_o", bufs=2,
                                 name=f"po{u}")
                    for i in range(8):
                        ex = ex_store.pop((u, i))
                        for j in range(2):
                            kb = 2 * i + j
                            blk = b * 16 + kb
                            nc.tensor.matmul(
                                po[:],
                                v_dual[:, blk * 130 + h * 65:
                                       blk * 130 + h * 65 + 65],
                                ex[:, j * TCH:(j + 1) * TCH],
                                start=(kb == 0), stop=(kb == S // 128 - 1))
                        if i + 2 < 8:
                            emit_s(u, i + 2)
                        elif u + 1 < len(sched):
                            emit_s(u + 1, i - 6)
                        if i == 0:
                            # deferred normalization of the previous unit:
                            # by now that pscr slot's exp is done, so the
                            # ps_s rotation slot for pbc is free
                            emit_tail()
                            if u in at_unit_start:
                                at_unit_start.pop(u)()
                        filler_slot()
                    # reciprocal right at unit end so the deferred pbc
                    # matmul never waits on the DVE
                    r65 = sb.tile([65, TCH], F32R, tag="r", bufs=2,
                                  name=f"r{u}")
                    nc.vector.reciprocal(r65[64:65, :], po[64:65, :])
                    pending_tail[0] = (b, h, qc, po, r65)

            def emit_a2a0():
                if collective:
                    nc.gpsimd.collective_compute(
                        "AllToAll", mybir.AluOpType.bypass,
                        replica_groups=[list(range(NCORES))],
                        ins=[bnc_in0[:]], outs=[bnc_out0[:]])
                else:
                    nc.sync.dma_start(bnc_out0[:], bnc_in0[:])

            def emit_a2a1(r):
                if collective:
                    nc.gpsimd.collective_compute(
                        "AllToAll", mybir.AluOpType.bypass,
                        replica_groups=[list(range(NCORES))],
                        ins=[bnc_in1[r][:]], outs=[bnc_out1[r][:]])
                else:
                    nc.sync.dma_start(bnc_out1[r][:], bnc_in1[r][:])

            # ---- output projection pass h: heads {4p+h, 4p+2+h};
            # oTf[h][j*64+r, p*512+t] = bnc_out[h][2p+j, r, t]
            oTf = {}

            def emit_oTf0():
                t = sb.tile([128, 4 * ROWS], BF16, tag="oTf", bufs=2,
                            name="oTf0")
                for j in range(2):
                    nc.sync.dma_start(
                        t[64 * j:64 * j + 64, :].rearrange(
                            "r (p tt) -> r p tt", tt=ROWS),
                        bnc_out0[:].rearrange(
                            "(p j) r tt -> j r p tt", j=2)[j])
                oTf[0] = t

            def emit_oTf1(r):
                if 1 not in oTf:
                    oTf[1] = sb.tile([128, 4 * ROWS], BF16, tag="oTf",
                                     bufs=2, name="oTf1")
                t = oTf[1]
                hw = ROWS // 2
                for j in range(2):
                    nc.sync.dma_start(
                        t[64 * j:64 * j + 64, :].rearrange(
                            "rr (p tt) -> rr p tt", tt=ROWS)[
                            :, :, r * hw:(r + 1) * hw],
                        bnc_out1[r][:].rearrange(
                            "(p j) rr tt -> j rr p tt", j=2)[j])

            o0sb = [sb.tile([128, 512], BF16, tag="o0sb", bufs=8,
                            name=f"o0sb{i}") for i in range(8)]

            def pass0_thunks(ci):
                sbi, doc = divmod(ci, 2)
                holder = {}

                def mk(p):
                    def t():
                        if p == 0:
                            holder["pout"] = ps.tile(
                                [128, 512], F32, tag="ps_a", bufs=2,
                                name=f"p0_{ci}")
                        nc.tensor.matmul(
                            holder["pout"][:],
                            oTf[0][:, p * ROWS + sbi * 128:
                                   p * ROWS + (sbi + 1) * 128],
                            wo_p[0][p][:, doc * 512:(doc + 1) * 512],
                            start=(p == 0), stop=(p == 3))
                        if p == 3:
                            nc.vector.tensor_copy(o0sb[ci][:],
                                                  holder["pout"][:])
                    return t
                return [(213, f"p0_{ci}", mk(p)) for p in range(4)]

            def emit_pass1_half(r):
                for sbi in (2 * r, 2 * r + 1):
                    outt = sb.tile([128, DO], BF16, tag="osb", bufs=2,
                                   name=f"outt{sbi}")
                    for doc in range(2):
                        pout = ps.tile([128, 512], F32, tag="ps_a", bufs=2,
                                       name=f"p1_{sbi}_{doc}")
                        for p in range(4):
                            nc.tensor.matmul(
                                pout[:],
                                oTf[1][:, p * ROWS + sbi * 128:
                                       p * ROWS + (sbi + 1) * 128],
                                wo_p[1][p][:, doc * 512:(doc + 1) * 512],
                                start=(p == 0), stop=(p == 3))
                        nc.vector.tensor_add(
                            outt[:, doc * 512:(doc + 1) * 512], pout[:],
                            o0sb[sbi * 2 + doc][:])
                        # ship each half as soon as its add lands so only
                        # the last 128KB DMA is exposed at the end
                        nc.sync.dma_start(
                            out_d[sbi * 128:(sbi + 1) * 128,
                                  doc * 512:(doc + 1) * 512],
                            outt[:, doc * 512:(doc + 1) * 512])

            # ---- schedule ----
            # Warm the PE clock while the first DMAs land (f32r with a
            # 64-wide moving AP runs at 4 cyc/row: ~394ns per warm matmul).
            for wi in range(12):
                wps = ps.tile([64, 64], F32, tag="ps_s", bufs=2,
                              name=f"swarm{wi}")
                nc.tensor.matmul(wps[:], ones_b[0:1, :], ones_b[0:1, :],
                                 start=True, stop=True)
            # chunks 0-1 in bulk, then attention starts (score block i of a
            # batch-0 unit only needs kT chunk i//2); chunks 2-7 and the
            # deferred q parts ration through the filler queue, with
            # just-in-time flush hooks at the score blocks that need them.
            for tci in range(2):
                emit_proj(tci, "qkv")
            for tci in range(2, 8):
                fill_q.extend(qk_part_thunks(tci, "k"))
                fill_q.extend(v_part_thunks(tci))
                fill_q.extend(qk_part_thunks(tci, "q"))

            # units 0 and 8 meet their later key chunks mid-stream (score
            # block i needs kT chunk i//2 resp. 4+i//2, AV block i needs
            # the matching v blocks); other units need their own q chunk
            # before their first scores
            def _fl(label):
                return lambda: flush_through(label)

            pre_s_hook[(0, 4)] = _fl("v2")
            pre_s_hook[(0, 6)] = _fl("v3")
            pre_s_hook[(2, 0)] = _fl("q2")
            pre_s_hook[(3, 0)] = _fl("q3")
            for qc in range(S // TCH):
                pre_s_hook[(8 + qc, 0)] = _fl(f"q{4 + qc}")

            def h0_done():
                # unit 11's obc has just been flushed by emit_tail above
                emit_a2a0()
                emit_oTf0()
                for ci in range(8):
                    fill_q.extend(pass0_thunks(ci))
            at_unit_start[12] = h0_done

            run_units()
            emit_tail()  # flush (1,1,3)'s obc before the h1 collectives
            flush_all()
            emit_a2a1(0)
            emit_a2a1(1)
            # warms (on a long-ready operand) bridge the PE clock across
            # the h1 collective window so the odd-heads projection pass
            # starts at 2.4 GHz
            for wi in range(32):
                wps = ps.tile([64, TCH], F32, tag="ps_s", bufs=2,
                              name=f"warm{wi}")
                nc.tensor.matmul(
                    wps[:], o0sb[0][:, 0:64], o0sb[0][:],
                    start=True, stop=True)
            emit_oTf1(0)
            emit_oTf1(1)
            emit_pass1_half(0)
            emit_pass1_half(1)

    nc.compile()
    return nc


def _get_nc():
    if "nc" not in _cache:
        _cache["nc"] = _build()
    return _cache["nc"]


def _dshuffle(w):
    # [D, 128] -> [128, 8*128] with out[p, dc*128+e] = w[dc*128+p, e]
    return w.reshape(8, 128, 128).transpose(1, 0, 2).reshape(128, 1024)


def _in_maps(x, Wq, Wk, Wv, Wo):
    bf16 = ml_dtypes.bfloat16
    xt = np.ascontiguousarray(
        x.reshape(T, D).T.astype(bf16))
    wo = np.ascontiguousarray(Wo.astype(bf16))
    maps = []
    for c in range(NCORES):
        h0, h1 = HPC * c, HPC * c + 1
        wqkv = np.concatenate(
            [_dshuffle(np.concatenate([W[h0], W[h1]], axis=1))
             for W in (Wq, Wk, Wv)], axis=1)
        maps.append({
            "xt": xt,
            "wqkv": np.ascontiguousarray(wqkv.astype(bf16)),
            "wo": wo,
        })
    return maps


def kernel(x, Wq, Wk, Wv, Wo, **_):
    nc = _get_nc()
    res = bass_utils.run_bass_kernel_spmd(
        nc, _in_maps(x, Wq, Wk, Wv, Wo), core_ids=list(range(NCORES)))
    out = np.concatenate(
        [res.results[c]["out"].astype(np.float32) for c in range(NCORES)],
        axis=0)
    return out.reshape(B, S, DO)


# revision 59
# speedup vs baseline: 1.0257x; 1.0010x over previous
"""Self-contained Trainium2 Bass kernel for the multi-head attention module.

Sharding: flat 8-way head tensor-parallelism. Core c owns heads {2c, 2c+1}
for both batches; after attention one 8-core AllToAll per head-pair index
reshards from head-space to sequence-space and each core runs the output
projection for its 512 token rows. Host concatenates the per-core row
chunks.

Layout: everything bf16 on the matmul paths (1 cyc/row on PE, half the
DMA + collective bytes). x is transposed on the host so the kernel DMAs
[D, T] tiles straight into SBUF: no PE transposes, no staging copies. V is
computed directly in [token, v] layout via xT-stationary matmuls. The
Activation engine runs only the softmax exps (it is the attention-phase
floor at ~1038ns per 256-key block vs the PE's 854ns); all PSUM->SBUF
copies live on DVE. Each attention unit is software-pipelined with scores
running two key-blocks ahead of the AV matmuls, and the projection /
output-pass matmuls are rationed into the ~184ns/block PE slack through a
filler queue so the Activation engine never starves. The output projection
is split into an even-heads pass (hidden behind late attention, after the
first AllToAll) and an odd-heads pass (the only work after the second
AllToAll).
"""

import sys

sys.path.insert(0, "/opt/trn_rl_repo")

from collections import deque

import ml_dtypes
import numpy as np

from concourse import bacc, bass_utils, mybir, tile

B, S, D, H, DK, DV, DO = 2, 2048, 1024, 16, 64, 64, 1024
T = B * S          # 4096 flattened tokens
NCORES = 8
HPC = H // NCORES  # 2 heads per core
ROWS = T // NCORES # 512 output rows per core
TCH = 512          # token chunk for projections / q chunks
F32 = mybir.dt.float32
F32R = mybir.dt.float32r
BF16 = mybir.dt.bfloat16
EXP = mybir.ActivationFunctionType.Exp

_cache = {}


def _build(collective=True):
    nc = bacc.Bacc("TRN2", target_bir_lowering=False, debug=False,
                   num_devices=NCORES if collective else 1)
    xt_d = nc.dram_tensor("xt", [D, T], BF16, kind="ExternalInput").ap()
    # host-pre-shuffled [128, 3*8*128]: wqkv[p, n*1024 + dc*128 + e] =
    # W_n[dc*128 + p, e] -- one fat contiguous DMA, no per-d-block strides
    wqkv_d = nc.dram_tensor("wqkv", [128, 3 * 8 * 128], BF16,
                            kind="ExternalInput").ap()
    wo_d = nc.dram_tensor("wo", [H * DV, DO], BF16, kind="ExternalInput").ap()
    out_d = nc.dram_tensor("out", [ROWS, DO], BF16, kind="ExternalOutput").ap()
    bnc_in0 = nc.dram_tensor("bnc_in0", [NCORES, 64, ROWS], BF16).ap()
    bnc_out0 = nc.dram_tensor("bnc_out0", [NCORES, 64, ROWS], BF16).ap()
    # the second head-pair's resharding is split into two row-half
    # collectives so the output projection can start on the first half
    # while the second is still in flight
    bnc_in1 = [nc.dram_tensor(f"bnc_in1_{r}", [NCORES, 64, ROWS // 2],
                              BF16).ap() for r in range(2)]
    bnc_out1 = [nc.dram_tensor(f"bnc_out1_{r}", [NCORES, 64, ROWS // 2],
                               BF16).ap() for r in range(2)]

    with tile.TileContext(nc) as tc:
        with (
            tc.tile_pool(name="sb", bufs=1) as sb,
            tc.tile_pool(name="ps", bufs=1, space="PSUM") as ps,
            nc.allow_low_precision(reason="bf16 compute is intentional"),
        ):
            # constants for the softmax-normalization broadcast matmul
            ones_f = sb.tile([128, 64], F32, tag="onesf", bufs=1)
            nc.vector.memset(ones_f[:], 1.0)
            ones_b = sb.tile([128, 64], F32R, tag="ones", bufs=1)
            nc.vector.tensor_copy(ones_b[:], ones_f[:])

            # HWDGE descriptor generation costs ~625ns per DMA instruction,
            # serialized, so inputs are fetched with as few fat strided DMAs
            # as possible. The first x chunk is interleaved with the weights
            # so phase 1 can start ~4us in.
            # qkv weights: one contiguous DMA for all three matrices
            # (host-pre-shuffled into d-block column layout)
            wqkv_sb = sb.tile([128, 3 * 8 * 128], BF16, tag="wqkv", bufs=1)
            _wn = {"q": 0, "k": 1, "v": 2}

            def w_slice(name, dc):
                c0 = _wn[name] * 1024 + dc * 128
                return wqkv_sb[:, c0:c0 + 128]

            # x^T: one [128, 8*512] tile per chunk (column blocks are the 8
            # d-blocks), loaded in two half DMAs (xc[p, dc*512+t] =
            # xt_d[dc*128+p, c0+t]).
            xTc = [sb.tile([128, 8 * TCH], BF16, tag="xTc", bufs=8,
                           name=f"xTc{tci}") for tci in range(8)]

            def load_x_chunk(tci, half, width=4):
                c0 = tci * TCH
                dc0 = half * width
                nc.sync.dma_start(
                    xTc[tci][:, dc0 * TCH:(dc0 + width) * TCH].rearrange(
                        "p (dc t) -> p dc t", t=TCH),
                    xt_d[dc0 * 128:(dc0 + width) * 128,
                         c0:c0 + TCH].rearrange(
                        "(dc p) t -> p dc t", p=128))

            # q weights first (phase 1 starts with the q projection), then
            # the first x chunk in quarters so its first d-blocks land
            # early, then the rest in halves
            nc.sync.dma_start(wqkv_sb[:, 0:1024], wqkv_d[:, 0:1024])
            for quarter in range(2):
                load_x_chunk(0, quarter, width=2)
            nc.sync.dma_start(wqkv_sb[:, 1024:3072], wqkv_d[:, 1024:3072])
            for quarter in range(2, 4):
                load_x_chunk(0, quarter, width=2)
            for tci in range(1, 8):
                load_x_chunk(tci, 0)
                load_x_chunk(tci, 1)

            def xT(dc, tci):
                return xTc[tci][:, dc * TCH:(dc + 1) * TCH]

            # wo pair tiles for the two projection passes: pass h reads heads
            # {4p+h, 4p+2+h} stacked on partitions, matching the oTf layout
            wo_p = {0: [], 1: []}
            for h in range(HPC):
                for p in range(4):
                    wt = sb.tile([128, DO], BF16, tag="wo", bufs=8,
                                 name=f"wo{h}_{p}")
                    for half, head in ((0, 4 * p + h), (1, 4 * p + 2 + h)):
                        nc.sync.dma_start(
                            wt[half * 64:half * 64 + 64, :],
                            wo_d[head * 64:head * 64 + 64, :])
                    wo_p[h].append(wt)

            # persistent activations
            qT = sb.tile([128, T], BF16, tag="qT", bufs=1)
            kT = sb.tile([128, T], BF16, tag="kT", bufs=1)
            # v in natural [token, v] layout: 32 t-blocks x (2 heads x
            # [64 v cols | ones]) -> AV stationary slices [128, 65]
            v_dual = sb.tile([128, 32 * 130], BF16, tag="vdual", bufs=1)
            ones_cols = v_dual[:].rearrange(
                "p (b h c) -> p b h c", h=2, c=65)[:, :, :, 64:65]
            nc.vector.memset(ones_cols, 1.0)

            last_obc = [None]

            # ---- filler queue: small PE thunks rationed into the
            # Act-bound slack of the attention inner loop ----
            fill_q = deque()  # entries: (cost_ns, label, thunk)

            def filler_slot(budget=200):
                spent = 0
                while fill_q and spent < budget:
                    cost, _, thunk = fill_q.popleft()
                    thunk()
                    spent += cost

            def flush_through(label):
                while any(e[1] == label for e in fill_q):
                    _, _, thunk = fill_q.popleft()
                    thunk()

            def flush_all():
                while fill_q:
                    fill_q.popleft()[2]()

            # ---- phase 1 parts: q/k (W stationary) and v (xT stationary)
            def qk_part_thunks(tci, name):
                holder = {}
                c0 = tci * TCH

                def mk(dc):
                    def t():
                        if dc == 0:
                            holder["pp"] = ps.tile(
                                [128, TCH], F32, tag="ps_a", bufs=2,
                                name=f"pp{tci}_{name}")
                        nc.tensor.matmul(
                            holder["pp"][:], w_slice(name, dc),
                            xT(dc, tci), start=(dc == 0), stop=(dc == 7))
                        if dc == 7:
                            dst = qT if name == "q" else kT
                            nc.vector.tensor_copy(dst[:, c0:c0 + TCH],
                                                  holder["pp"][:])
                    return t
                return [(213, f"{name}{tci}", mk(dc)) for dc in range(8)]

            def v_part_thunks(tci):
                holder = {}

                def mk(tb, dc):
                    def t():
                        if tb == 0 and dc == 0:
                            holder["pv"] = ps.tile(
                                [128, TCH], F32, tag="ps_a", bufs=2,
                                name=f"pv{tci}")
                        nc.tensor.matmul(
                            holder["pv"][:, tb * 128:(tb + 1) * 128],
                            xTc[tci][:, dc * TCH + tb * 128:
                                      dc * TCH + (tb + 1) * 128],
                            w_slice("v", dc),
                            start=(dc == 0), stop=(dc == 7))
                        if tb == 3 and dc == 7:
                            vd = v_dual[:, tci * 4 * 130:
                                        (tci + 1) * 4 * 130].rearrange(
                                "p (b h c) -> p b h c", h=2, c=65)[
                                :, :, :, 0:64]
                            nc.vector.tensor_copy(
                                vd, holder["pv"][:].rearrange(
                                    "p (b h c) -> p b h c", h=2, c=64))
                    return t
                return [(60, f"v{tci}", mk(tb, dc))
                        for tb in range(4) for dc in range(8)]

            def emit_proj(tci, which):
                # bulk emission (used for the pre-attention chunks); q
                # first to match the input DMA queue order
                if "q" in which:
                    for e in qk_part_thunks(tci, "q"):
                        e[2]()
                if "k" in which:
                    for e in qk_part_thunks(tci, "k"):
                        e[2]()
                if "v" in which:
                    for e in v_part_thunks(tci):
                        e[2]()

            # ---- attention unit (batch, head, q-chunk), software-pipelined:
            # scores+exp run two key-blocks ahead of the AV matmuls, filler
            # thunks absorb the ~184ns/block PE slack, and the
            # normalization tail is deferred into the next unit so the PE
            # never head-of-line blocks on the DVE reciprocal.
            pending_tail = [None]

            def emit_tail():
                if pending_tail[0] is None:
                    return
                b, h, qc, po, r65 = pending_tail[0]
                pending_tail[0] = None
                pbc = ps.tile([64, TCH], F32, tag="ps_s", bufs=2,
                              name=f"pbc{b}_{h}_{qc}")
                nc.tensor.matmul(pbc[:], ones_b[64:65, :],
                                 r65[64:65, :], start=True, stop=True)
                bc_sb = sb.tile([64, TCH], F32R, tag="bcsb", bufs=2,
                                name=f"bcsb{b}_{h}_{qc}")
                nc.vector.tensor_copy(bc_sb[:], pbc[:])
                obc = sb.tile([64, TCH], BF16, tag="obc", bufs=3,
                              name=f"obc{b}_{h}_{qc}")
                nc.vector.tensor_mul(obc[:], po[0:64, :], bc_sb[:])
                shard = b * (S // TCH) + qc
                if h == 0:
                    nc.sync.dma_start(bnc_in0[shard, :, :], obc[:])
                else:
                    for r in range(2):
                        nc.sync.dma_start(
                            bnc_in1[r][shard, :, :],
                            obc[:, r * (TCH // 2):(r + 1) * (TCH // 2)])
                last_obc[0] = obc

            # The 16 attention units run as one globally software-pipelined
            # stream: the scores+exp block always runs exactly two
            # key-blocks ahead of the AV matmuls (which matches the 2-deep
            # ps_s rotation: a new pscr's slot belongs to the block whose
            # ex the PE just consumed), so the PE never waits on an exp --
            # not even across unit boundaries.
            sched = ([(0, h, qc) for h in range(HPC)
                      for qc in range(S // TCH)]
                     + [(1, 0, qc) for qc in range(S // TCH)]
                     + [(1, 1, qc) for qc in range(S // TCH)])
            ex_store = {}
            pre_s_hook = {}

            def emit_s(u, i):
                if (u, i) in pre_s_hook:
                    pre_s_hook.pop((u, i))()
                b, h, qc = sched[u]
                qoff = b * S + qc * TCH
                pscr = ps.tile([128, 2 * TCH], F32, tag="ps_s", bufs=2,
                               name=f"pscr{u}_{i}")
                for j in range(2):
                    koff = b * S + (2 * i + j) * 128
                    nc.tensor.matmul(
                        pscr[:, j * TCH:(j + 1) * TCH],
                        kT[h * 64:(h + 1) * 64, koff:koff + 128],
                        qT[h * 64:(h + 1) * 64, qoff:qoff + TCH],
                        start=True, stop=True)
                ex = sb.tile([128, 2 * TCH], BF16, tag="ex", bufs=4,
                             name=f"ex{u}_{i}")
                nc.scalar.activation(ex[:], pscr[:], EXP, scale=0.125)
                ex_store[(u, i)] = ex

            at_unit_start = {}

            def run_units():
                emit_s(0, 0)
                emit_s(0, 1)
                for u in range(len(sched)):
                    b, h, qc = sched[u]
                    po = ps.tile([65, TCH], F32, tag="ps_o", bufs=2,
                                 name=f"po{u}")
                    for i in range(8):
                        ex = ex_store.pop((u, i))
                        for j in range(2):
                            kb = 2 * i + j
                            blk = b * 16 + kb
                            nc.tensor.matmul(
                                po[:],
                                v_dual[:, blk * 130 + h * 65:
                                       blk * 130 + h * 65 + 65],
                                ex[:, j * TCH:(j + 1) * TCH],
                                start=(kb == 0), stop=(kb == S // 128 - 1))
                        if i + 2 < 8:
                            emit_s(u, i + 2)
                        elif u + 1 < len(sched):
                            emit_s(u + 1, i - 6)
                        if i == 0:
                            # deferred normalization of the previous unit:
                            # by now that pscr slot's exp is done, so the
                            # ps_s rotation slot for pbc is free
                            emit_tail()
                            if u in at_unit_start:
                                at_unit_start.pop(u)()
                        filler_slot()
                    # reciprocal right at unit end so the deferred pbc
                    # matmul never waits on the DVE
                    r65 = sb.tile([65, TCH], F32R, tag="r", bufs=2,
                                  name=f"r{u}")
                    nc.vector.reciprocal(r65[64:65, :], po[64:65, :])
                    pending_tail[0] = (b, h, qc, po, r65)

            def emit_a2a0():
                if collective:
                    nc.gpsimd.collective_compute(
                        "AllToAll", mybir.AluOpType.bypass,
                        replica_groups=[list(range(NCORES))],
                        ins=[bnc_in0[:]], outs=[bnc_out0[:]])
                else:
                    nc.sync.dma_start(bnc_out0[:], bnc_in0[:])

            def emit_a2a1(r):
                if collective:
                    nc.gpsimd.collective_compute(
                        "AllToAll", mybir.AluOpType.bypass,
                        replica_groups=[list(range(NCORES))],
                        ins=[bnc_in1[r][:]], outs=[bnc_out1[r][:]])
                else:
                    nc.sync.dma_start(bnc_out1[r][:], bnc_in1[r][:])

            # ---- output projection pass h: heads {4p+h, 4p+2+h};
            # oTf[h][j*64+r, p*512+t] = bnc_out[h][2p+j, r, t]
            oTf = {}

            def emit_oTf0():
                t = sb.tile([128, 4 * ROWS], BF16, tag="oTf", bufs=2,
                            name="oTf0")
                for j in range(2):
                    nc.sync.dma_start(
                        t[64 * j:64 * j + 64, :].rearrange(
                            "r (p tt) -> r p tt", tt=ROWS),
                        bnc_out0[:].rearrange(
                            "(p j) r tt -> j r p tt", j=2)[j])
                oTf[0] = t

            def emit_oTf1(r):
                if 1 not in oTf:
                    oTf[1] = sb.tile([128, 4 * ROWS], BF16, tag="oTf",
                                     bufs=2, name="oTf1")
                t = oTf[1]
                hw = ROWS // 2
                for j in range(2):
                    nc.sync.dma_start(
                        t[64 * j:64 * j + 64, :].rearrange(
                            "rr (p tt) -> rr p tt", tt=ROWS)[
                            :, :, r * hw:(r + 1) * hw],
                        bnc_out1[r][:].rearrange(
                            "(p j) rr tt -> j rr p tt", j=2)[j])

            o0sb = [sb.tile([128, 512], BF16, tag="o0sb", bufs=8,
                            name=f"o0sb{i}") for i in range(8)]

            def pass0_thunks(ci):
                sbi, doc = divmod(ci, 2)
                holder = {}

                def mk(p):
                    def t():
                        if p == 0:
                            holder["pout"] = ps.tile(
                                [128, 512], F32, tag="ps_a", bufs=2,
                                name=f"p0_{ci}")
                        nc.tensor.matmul(
                            holder["pout"][:],
                            oTf[0][:, p * ROWS + sbi * 128:
                                   p * ROWS + (sbi + 1) * 128],
                            wo_p[0][p][:, doc * 512:(doc + 1) * 512],
                            start=(p == 0), stop=(p == 3))
                        if p == 3:
                            nc.vector.tensor_copy(o0sb[ci][:],
                                                  holder["pout"][:])
                    return t
                return [(213, f"p0_{ci}", mk(p)) for p in range(4)]

            def emit_pass1_half(r):
                for sbi in (2 * r, 2 * r + 1):
                    outt = sb.tile([128, DO], BF16, tag="osb", bufs=2,
                                   name=f"outt{sbi}")
                    for doc in range(2):
                        pout = ps.tile([128, 512], F32, tag="ps_a", bufs=2,
                                       name=f"p1_{sbi}_{doc}")
                        for p in range(4):
                            nc.tensor.matmul(
                                pout[:],
                                oTf[1][:, p * ROWS + sbi * 128:
                                       p * ROWS + (sbi + 1) * 128],
                                wo_p[1][p][:, doc * 512:(doc + 1) * 512],
                                start=(p == 0), stop=(p == 3))
                        nc.vector.tensor_add(
                            outt[:, doc * 512:(doc + 1) * 512], pout[:],
                            o0sb[sbi * 2 + doc][:])
                        # ship each half as soon as its add lands so only
                        # the last 128KB DMA is exposed at the end
                        nc.sync.dma_start(
                            out_d[sbi * 128:(sbi + 1) * 128,
                                  doc * 512:(doc + 1) * 512],
                            outt[:, doc * 512:(doc + 1) * 512])

            # ---- schedule ----
            # Warm the PE clock while the first DMAs land (f32r with a
            # 64-wide moving AP runs at 4 cyc/row: ~394ns per warm matmul).
            for wi in range(12):
                wps = ps.tile([64, 64], F32, tag="ps_s", bufs=2,
                              name=f"swarm{wi}")
                nc.tensor.matmul(wps[:], ones_b[0:1, :], ones_b[0:1, :],
                                 start=True, stop=True)
            # chunks 0-1 in bulk, then attention starts (score block i of a
            # batch-0 unit only needs kT chunk i//2); chunks 2-7 and the
            # deferred q parts ration through the filler queue, with
            # just-in-time flush hooks at the score blocks that need them.
            for tci in range(2):
                emit_proj(tci, "qkv")
            for tci in range(2, 8):
                fill_q.extend(qk_part_thunks(tci, "k"))
                fill_q.extend(v_part_thunks(tci))
                fill_q.extend(qk_part_thunks(tci, "q"))

            # units 0 and 8 meet their later key chunks mid-stream (score
            # block i needs kT chunk i//2 resp. 4+i//2, AV block i needs
            # the matching v blocks); other units need their own q chunk
            # before their first scores
            def _fl(label):
                return lambda: flush_through(label)

            pre_s_hook[(0, 4)] = _fl("v2")
            pre_s_hook[(0, 6)] = _fl("v3")
            pre_s_hook[(2, 0)] = _fl("q2")
            pre_s_hook[(3, 0)] = _fl("q3")
            for qc in range(S // TCH):
                pre_s_hook[(8 + qc, 0)] = _fl(f"q{4 + qc}")

            def h0_done():
                # unit 11's obc has just been flushed by emit_tail above
                emit_a2a0()
                emit_oTf0()
                for ci in range(8):
                    fill_q.extend(pass0_thunks(ci))
            at_unit_start[12] = h0_done

            run_units()
            emit_tail()  # flush (1,1,3)'s obc before the h1 collectives
            flush_all()
            emit_a2a1(0)
            emit_a2a1(1)
            # warms (on a long-ready operand) bridge the PE clock across
            # the h1 collective window so the odd-heads projection pass
            # starts at 2.4 GHz
            for wi in range(32):
                wps = ps.tile([64, TCH], F32, tag="ps_s", bufs=2,
                              name=f"warm{wi}")
                nc.tensor.matmul(
                    wps[:], o0sb[0][:, 0:64], o0sb[0][:],
                    start=True, stop=True)
            emit_oTf1(0)
            emit_oTf1(1)
            emit_pass1_half(0)
            emit_pass1_half(1)

    nc.compile()
    return nc


def _get_nc():
    if "nc" not in _cache:
        _cache["nc"] = _build()
    return _cache["nc"]


def _dshuffle(w):
    # [D, 128] -> [128, 8*128] with out[p, dc*128+e] = w[dc*128+p, e]
    return w.reshape(8, 128, 128).transpose(1, 0, 2).reshape(128, 1024)


def _in_maps(x, Wq, Wk, Wv, Wo):
    bf16 = ml_dtypes.bfloat16
    xt = np.ascontiguousarray(
        x.reshape(T, D).T.astype(bf16))
    wo = np.ascontiguousarray(Wo.astype(bf16))
    maps = []
    for c in range(NCORES):
        h0, h1 = HPC * c, HPC * c + 1
        wqkv = np.concatenate(
            [_dshuffle(np.concatenate([W[h0], W[h1]], axis=1))
             for W in (Wq, Wk, Wv)], axis=1)
        maps.append({
            "xt": xt,
            "wqkv": np.ascontiguousarray(wqkv.astype(bf16)),
            "wo": wo,
        })
    return maps


def kernel(x, Wq, Wk, Wv, Wo, **_):
    nc = _get_nc()
    res = bass_utils.run_bass_kernel_spmd(
        nc, _in_maps(x, Wq, Wk, Wv, Wo), core_ids=list(range(NCORES)))
    out = np.concatenate(
        [res.results[c]["out"].astype(np.float32) for c in range(NCORES)],
        axis=0)
    return out.reshape(B, S, DO)
